# revision 1
# baseline (speedup 1.0000x reference)
"""Multi-head attention (B=2,S=4096,E=768,H=12,D=64 + 16-token K/V prompt
prefix) on 8 Trainium2 NeuronCores.

Sharding: 2 batches x 4 head-groups (3 heads each). Each core computes QKV
projections for its 3 heads, full attention over its batch, and a partial
output projection (its 192 ctx channels); the host sums the 4 partials per
batch.

Per-core kernel layout (all attention math in "transposed" orientation so no
on-chip transposes are needed):
  qT[c,s]   = Wq_g @ query^T           (lhsT=Wq_g^T chunks, rhs=queryT chunks)
  kT[c,s]   likewise; prompt K prefix DMA'd in pre-transposed from host
  v[s,c]    natural orientation        (lhsT=valueT chunks, rhs=Wv_g^T)
  scoresT[k,q] = kT^T-slices @ qT      (lhsT=kT tile [64,128], rhs=qT [64,512])
  expT = Exp(scoresT / sqrt(D))        (ScalarE, reads PSUM directly)
  ctxT[d,q](+denom row) = v_aug^T @ expT  (v_aug has a ones column -> row 64
                                           accumulates the softmax denominator)
  ctxT_norm = ctxT * bcast(1/denom)    (fused into PSUM evacuation)
  outT[e,q] partial = Wo_g^T-slices @ ctxT_norm

Pipelining: one global software-pipelined slot stream over (sq, h, kt).
Scores matmuls + exp lead; ctx matmuls trail by TRAIL slots; V-projection
matmuls ride the first 32 slots; the partial out-projection for a q-block is
emitted as soon as its last head is normalized.  ScalarE (exp) is the
bottleneck engine and is kept ~100% busy.
"""

import sys
import threading

import numpy as np

if "/opt/trn_rl_repo" not in sys.path:
    sys.path.insert(0, "/opt/trn_rl_repo")

import ml_dtypes

BF16 = ml_dtypes.bfloat16

B, S, E, H, D, PP = 2, 4096, 768, 12, 64, 16
NCORES = 8
NG = 4          # head-groups (tensor parallel)
HL = H // NG    # 3 local heads
CL = HL * D     # 192 local channels
SKV = PP + S    # 4112
NKT = S // 128  # 32 full k-tiles (prefix handled separately)
QT = 1024       # q tile width for scores/exp/ctx
NSQ = S // QT   # 4
TRAIL = 12       # ctx matmuls trail scores by this many slots
NST = S // 128  # 32 v stiles

_lock = threading.Lock()
_compiled = {}


def _build():
    import concourse.bass as bass  # noqa: F401
    import concourse.mybir as mybir
    import concourse.tile as tile
    from concourse import bacc

    f32 = mybir.dt.float32
    bf16 = mybir.dt.bfloat16
    EXP = mybir.ActivationFunctionType.Exp

    nc = bacc.Bacc("TRN2", target_bir_lowering=False, debug=False)

    xqT = nc.dram_tensor("xqT", [E, S], bf16, kind="ExternalInput").ap()
    xkT = nc.dram_tensor("xkT", [E, S], bf16, kind="ExternalInput").ap()
    xvT = nc.dram_tensor("xvT", [E, S], bf16, kind="ExternalInput").ap()
    wqT = nc.dram_tensor("wqT", [E, CL], bf16, kind="ExternalInput").ap()
    wkT = nc.dram_tensor("wkT", [E, CL], bf16, kind="ExternalInput").ap()
    wvT = nc.dram_tensor("wvT", [E, CL], bf16, kind="ExternalInput").ap()
    woT = nc.dram_tensor("woT", [CL, E], bf16, kind="ExternalInput").ap()
    bq = nc.dram_tensor("bq", [CL, 1], f32, kind="ExternalInput").ap()
    bk = nc.dram_tensor("bk", [CL, 1], f32, kind="ExternalInput").ap()
    bv = nc.dram_tensor("bv", [1, CL], f32, kind="ExternalInput").ap()
    kpT = nc.dram_tensor("kpT", [128, 2, PP], bf16, kind="ExternalInput").ap()
    vp = nc.dram_tensor("vp", [PP, HL, D + 1], bf16, kind="ExternalInput").ap()
    outT = nc.dram_tensor("outT", [E, S], f32, kind="ExternalOutput").ap()

    with tile.TileContext(nc) as tc:
        with tc.tile_pool(name="persist", bufs=1) as pers:
            # q-projection weights/bias first: they gate the very first
            # matmuls, so don't queue them behind the other ~1MB of DMAs
            wq_sb = pers.tile([128, 6, CL], bf16)
            nc.sync.dma_start(wq_sb[:], wqT.rearrange("(t p) c -> p t c", p=128))
            bq_sb = pers.tile([128, 2], f32)
            nc.sync.dma_start(bq_sb[:, 0:1], bq[0:128, :])
            nc.sync.dma_start(bq_sb[0:64, 1:2], bq[128:CL, :])

            wk_sb = pers.tile([128, 6, CL], bf16)
            wv_sb = pers.tile([128, 6, CL], bf16)
            wo_sb = pers.tile([128, 2, E], bf16)
            bk_sb = pers.tile([128, 2], f32)
            bvb_sb = pers.tile([128, CL], f32)
            kpT_sb = pers.tile([128, 2, PP], bf16)
            vp_sb = pers.tile([PP, HL, D + 1], bf16)

            # activations (all bf16)
            qT_sb = pers.tile([128, 2, S], bf16)
            kT_sb = pers.tile([128, 2, S], bf16)   # no prefix; kpT separate
            v_sb = pers.tile([128, NST, HL, D + 1], bf16)
            ctxT_sb = pers.tile([128, 2, S], bf16)
            expp_sb = pers.tile([PP, HL, S], bf16)  # prefix exp rows per head

            nc.vector.memset(v_sb[:, :, :, D:D + 1], 1.0)

            # ---------------- Phase 1a: Q / K projections ----------------
            # PE-bound prologue; ScalarE is idle here by design --
            # nothing downstream can run before qT/kT exist.
            with (
                tc.tile_pool(name="ps_proj", bufs=2, space="PSUM") as pp,
                tc.tile_pool(name="xq_pool", bufs=4) as xq_pool,
            ):
                def proj_block(xin, wsb, bsb, dst, sq, eng=None):
                    eng = eng or nc.sync
                    p0 = pp.tile([128, QT], f32, tag="p0", name="p0")
                    p1 = pp.tile([64, QT], f32, tag="p1", name="p1")
                    for ech in range(6):
                        xt = xq_pool.tile([128, QT], bf16, tag="xt",
                                          name="xt")
                        eng.dma_start(
                            xt[:],
                            xin[ech * 128:(ech + 1) * 128,
                                sq * QT:(sq + 1) * QT],
                        )
                        for n in range(QT // 512):
                            ns = slice(n * 512, (n + 1) * 512)
                            nc.tensor.matmul(
                                p0[:, ns], wsb[:, ech, 0:128], xt[:, ns],
                                start=(ech == 0), stop=(ech == 5),
                            )
                            nc.tensor.matmul(
                                p1[:, ns], wsb[:, ech, 128:CL], xt[:, ns],
                                start=(ech == 0), stop=(ech == 5),
                            )
                    ds = slice(sq * QT, (sq + 1) * QT)
                    nc.vector.tensor_scalar_add(
                        dst[:, 0, ds], p0[:], bsb[:, 0:1])
                    nc.vector.tensor_scalar_add(
                        dst[0:64, 1, ds], p1[:], bsb[0:64, 1:2])

                proj_block(xqT, wq_sb, bq_sb, qT_sb, 0)
                # now that the critical q DMAs are queued, stream in the
                # remaining weights behind them
                nc.sync.dma_start(
                    wk_sb[:], wkT.rearrange("(t p) c -> p t c", p=128))
                nc.sync.dma_start(bk_sb[:, 0:1], bk[0:128, :])
                nc.sync.dma_start(bk_sb[0:64, 1:2], bk[128:CL, :])
                nc.sync.dma_start(kpT_sb[:], kpT[:])
                nc.sync.dma_start(
                    wv_sb[:], wvT.rearrange("(t p) c -> p t c", p=128))
                nc.sync.dma_start(bvb_sb[:], bv.to_broadcast((128, CL)))
                nc.sync.dma_start(vp_sb[:], vp[:])
                nc.sync.dma_start(wo_sb[:, 0, :], woT[0:128, :])
                nc.sync.dma_start(wo_sb[0:64, 1, :], woT[128:CL, :])

                # sq0 prompt-prefix scores+exp only need qT(sq0)+kpT: emit
                # them before the K projections so ScalarE starts (and pays
                # the one-time exp table load) ~25us earlier.  psp borrows a
                # ps_proj "p1" slot, so no extra PSUM pressure.
                for h in range(HL):
                    pr, po = h // 2, 64 * (h % 2)
                    psp0 = pp.tile([PP, QT], f32, tag="p1", name="psp0")
                    for n in range(QT // 512):
                        ns = slice(n * 512, (n + 1) * 512)
                        nc.tensor.matmul(
                            psp0[:, ns],
                            kpT_sb[po:po + 64, pr, :],
                            qT_sb[po:po + 64, pr, ns],
                            start=True, stop=True,
                        )
                    nc.scalar.activation(
                        expp_sb[:, h, 0:QT], psp0[:],
                        EXP, scale=float(D) ** -0.5,
                    )

                for sq in range(NSQ):
                    proj_block(xkT, wk_sb, bk_sb, kT_sb, sq)

            # ---------- attention + V-proj + out-proj: one slot stream ----------
            with (
                tc.tile_pool(name="ps_s", bufs=2, space="PSUM") as ps_s,
                tc.tile_pool(name="ps_c", bufs=1, space="PSUM") as ps_c,
                tc.tile_pool(name="ps_sm", bufs=2, space="PSUM") as ps_sm,
                tc.tile_pool(name="expt_pool", bufs=20) as expt_pool,
                tc.tile_pool(name="nrm_pool", bufs=2) as nrm_pool,
                tc.tile_pool(name="xv_pool", bufs=8) as xv_pool,
                tc.tile_pool(name="xq2_pool", bufs=7) as xq2_pool,
                tc.tile_pool(name="out_pool", bufs=4) as out_pool,
            ):
                def emit_prefix(sq, h):
                    pr, po = h // 2, 64 * (h % 2)
                    psp = ps_s.tile([PP, QT], f32, tag="pss", name="psp")
                    for n in range(QT // 512):
                        ns = slice(n * 512, (n + 1) * 512)
                        qs = slice(sq * QT + n * 512, sq * QT + (n + 1) * 512)
                        nc.tensor.matmul(
                            psp[:, ns],
                            kpT_sb[po:po + 64, pr, :],
                            qT_sb[po:po + 64, pr, qs],
                            start=True, stop=True,
                        )
                    nc.scalar.activation(
                        expp_sb[:, h, sq * QT:(sq + 1) * QT], psp[:],
                        EXP, scale=float(D) ** -0.5,
                    )

                # Background q-projection for sq 1..3 (op-granular, drained
                # one op per stream slot using the time-multiplexed sm pool)
                def make_bg_qproj(sq):
                    ops = []
                    state = {}

                    def dma_op():
                        tiles = []
                        for ech in range(6):
                            xt2 = xq2_pool.tile([128, QT], bf16, tag="xt2",
                                                name="xt2")
                            nc.sync.dma_start(
                                xt2[:],
                                xqT[ech * 128:(ech + 1) * 128,
                                    sq * QT:(sq + 1) * QT],
                            )
                            tiles.append(xt2)
                        state["xt"] = tiles

                    ops.append(dma_op)

                    def mk_mm(c, grp, ech):
                        def op():
                            if ech == 0:
                                state[(c, grp)] = ps_sm.tile(
                                    [128, 512], f32, tag="sm", name="pq")
                            pt = state[(c, grp)]
                            rows = 128 if grp == 0 else 64
                            wc = slice(0, 128) if grp == 0 else slice(128, CL)
                            nc.tensor.matmul(
                                pt[0:rows, :], wq_sb[:, ech, wc],
                                state["xt"][ech][:, c * 512:(c + 1) * 512],
                                start=(ech == 0), stop=(ech == 5),
                            )
                        return op

                    def mk_evac(c, grp):
                        def op():
                            pt = state.pop((c, grp))
                            qs = slice(sq * QT + c * 512,
                                       sq * QT + (c + 1) * 512)
                            if grp == 0:
                                nc.vector.tensor_scalar_add(
                                    qT_sb[:, 0, qs], pt[:, :], bq_sb[:, 0:1])
                            else:
                                nc.vector.tensor_scalar_add(
                                    qT_sb[0:64, 1, qs], pt[0:64, :],
                                    bq_sb[0:64, 1:2])
                        return op

                    for c in range(QT // 512):
                        for grp in range(2):
                            for ech in range(6):
                                ops.append(mk_mm(c, grp, ech))
                            ops.append(mk_evac(c, grp))
                    for h in range(HL):
                        ops.append(lambda h=h: emit_prefix(sq, h))
                    return ops

                bg_work = []
                for nb, sqb in ((32, 1), (66, 2), (150, 3)):
                    for op in make_bg_qproj(sqb):
                        bg_work.append((nb, op))

                # xv DMA loads, one sq-group of 6 chunks at a time
                xvts = {}

                def load_xv(sqx):
                    tiles = []
                    for ech in range(6):
                        xvt = xv_pool.tile([128, QT], bf16, tag="xvt",
                                           name="xvt")
                        nc.sync.dma_start(
                            xvt[:],
                            xvT[ech * 128:(ech + 1) * 128,
                                sqx * QT:(sqx + 1) * QT],
                        )
                        tiles.append(xvt)
                    xvts[sqx] = tiles

                def emit_vproj(st):
                    sqx, stl = st // (QT // 128), st % (QT // 128)
                    if st == 0:
                        load_xv(0)
                    if stl == 0 and sqx + 1 < NSQ:
                        load_xv(sqx + 1)
                    pv = ps_sm.tile([128, 512], f32, tag="sm", name="pv")
                    for ech in range(6):
                        nc.tensor.matmul(
                            pv[:, 0:CL],
                            xvts[sqx][ech][:, stl * 128:(stl + 1) * 128],
                            wv_sb[:, ech, :],
                            start=(ech == 0), stop=(ech == 5),
                        )
                    nc.vector.tensor_add(
                        v_sb[:, st, :, 0:D],
                        pv[:, 0:CL].rearrange("p (h d) -> p h d", h=HL),
                        bvb_sb[:].rearrange("p (h d) -> p h d", h=HL),
                    )
                    if stl == (QT // 128) - 1:
                        del xvts[sqx]

                def emit_scores_exp(sq, h, kt):
                    pr, po = h // 2, 64 * (h % 2)
                    lhsT_k = kT_sb[po:po + 64, pr, kt * 128:(kt + 1) * 128]
                    pss = ps_s.tile([128, QT], f32, tag="pss", name="pss")
                    expt = expt_pool.tile([128, QT], bf16, tag="expt",
                                          name="expt")
                    for n in range(QT // 512):
                        ns = slice(n * 512, (n + 1) * 512)
                        qs = slice(sq * QT + n * 512, sq * QT + (n + 1) * 512)
                        nc.tensor.matmul(
                            pss[:, ns], lhsT_k, qT_sb[po:po + 64, pr, qs],
                            start=True, stop=True,
                        )
                    nc.scalar.activation(
                        expt[:], pss[:], EXP, scale=float(D) ** -0.5,
                    )
                    return expt

                psc_tiles = {}

                def emit_ctx(sq, h, kt, expt):
                    key = (sq, h)
                    if kt == 0:
                        psc_tiles[key] = ps_c.tile([D + 1, QT], f32,
                                                   tag="psc", name="psc")
                    psc = psc_tiles[key]
                    for n in range(QT // 512):
                        ns = slice(n * 512, (n + 1) * 512)
                        nc.tensor.matmul(
                            psc[:, ns], v_sb[:, kt, h, :], expt[:, ns],
                            start=(kt == 0), stop=(kt == NKT - 1),
                        )
                    if kt == TRAIL - 1:
                        # prompt-prefix ctx contribution (reads expp_sb rows)
                        for n in range(QT // 512):
                            ns = slice(n * 512, (n + 1) * 512)
                            qs = slice(sq * QT + n * 512,
                                       sq * QT + (n + 1) * 512)
                            nc.tensor.matmul(
                                psc[:, ns], vp_sb[:, h, :],
                                expp_sb[:, h, qs],
                                start=False, stop=False,
                            )
                    if kt == NKT - 1:
                        emit_norm(sq, h, psc)
                        del psc_tiles[key]

                def emit_norm(sq, h, psc):
                    pr, po = h // 2, 64 * (h % 2)
                    rc = nrm_pool.tile([1, QT], f32, tag="rc", name="rc")
                    nc.vector.reciprocal(rc[:], psc[D:D + 1, :])
                    rb = nrm_pool.tile([64, QT], f32, tag="rb", name="rb")
                    nc.gpsimd.partition_broadcast(rb[:], rc[:])
                    nc.vector.tensor_mul(
                        ctxT_sb[po:po + 64, pr, sq * QT:(sq + 1) * QT],
                        psc[0:D, :], rb[:],
                    )

                outproj_work = []

                def emit_outproj(sq):
                    # queue the 12 out-projection tiles; drained 1/slot so
                    # they never lump up in front of scores matmuls
                    for et in range(6):
                        for n in range(QT // 512):
                            outproj_work.append((et, sq * 2 + n))

                def emit_outproj_tile(et, qn):
                    es = slice(et * 128, (et + 1) * 128)
                    qs = slice(qn * 512, (qn + 1) * 512)
                    po3 = ps_sm.tile([128, 512], f32, tag="sm", name="po3")
                    nc.tensor.matmul(
                        po3[:], wo_sb[:, 0, es], ctxT_sb[:, 0, qs],
                        start=True, stop=False,
                    )
                    nc.tensor.matmul(
                        po3[:], wo_sb[0:64, 1, es], ctxT_sb[0:64, 1, qs],
                        start=False, stop=True,
                    )
                    ot = out_pool.tile([128, 512], f32, tag="ot", name="ot")
                    nc.vector.tensor_copy(ot[:], po3[:])
                    nc.sync.dma_start(outT[es, qs], ot[:])

                slots = [(sq, h, kt)
                         for sq in range(NSQ)
                         for h in range(HL)
                         for kt in range(NKT)]
                # ctx trails scores by TRAIL slots; a block-opening ctx
                # (kt==0, start=True) trails by TRAIL+GAP so the previous
                # block's norm chain (recip -> bcast -> mul, ~4us) can free
                # the single psc slot without stalling the PE queue.  The
                # stream catches back up popping 2 ctxs per slot.
                GAP = 6
                pending = []

                def pop_one():
                    (s2, e2) = pending.pop(0)
                    emit_ctx(*s2, e2)
                    if s2[2] == NKT - 1 and s2[1] == HL - 1:
                        emit_outproj(s2[0])

                vst = 0
                for j, slot in enumerate(slots):
                    # scores matmuls first in each slot so exp (the
                    # bottleneck engine's feed) is never queue-delayed
                    expt = emit_scores_exp(*slot)
                    pending.append((slot, expt))
                    if vst < NST:
                        emit_vproj(vst)
                        vst += 1
                    # near the stream end the trail no longer buys ScalarE
                    # slack -- drain it so the final norm/out-proj/store
                    # chain starts as early as possible
                    trail_eff = TRAIL if j < len(slots) - 34 else 2
                    for _ in range(3):
                        if not pending:
                            break
                        need = (trail_eff + GAP if pending[0][0][2] == 0
                                else trail_eff)
                        if len(pending) > need:
                            pop_one()
                        else:
                            break
                    if bg_work and j >= bg_work[0][0]:
                        bg_work.pop(0)[1]()
                    elif outproj_work:
                        emit_outproj_tile(*outproj_work.pop(0))
                while pending:
                    pop_one()
                    if outproj_work:
                        emit_outproj_tile(*outproj_work.pop(0))
                for _, op in bg_work:
                    op()
                while outproj_work:
                    emit_outproj_tile(*outproj_work.pop(0))

    nc.compile()
    return nc


def _get_nc():
    with _lock:
        if "nc" not in _compiled:
            _compiled["nc"] = _build()
        return _compiled["nc"]


def _prep_in_maps(query, key, value, prompt, Wq, bq, Wk, bk, Wv, bv, Wo, bo):
    f32 = np.float32
    qT = [np.ascontiguousarray(query[b].T).astype(BF16) for b in range(B)]
    kT = [np.ascontiguousarray(key[b].T).astype(BF16) for b in range(B)]
    vT = [np.ascontiguousarray(value[b].T).astype(BF16) for b in range(B)]
    in_maps = []
    for core in range(NCORES):
        b, g = core // NG, core % NG
        cs = slice(g * CL, (g + 1) * CL)
        kp = np.zeros((128, 2, PP), BF16)
        vpa = np.zeros((PP, HL, D + 1), BF16)
        vpa[:, :, D] = 1.0
        for h in range(HL):
            gh = g * HL + h
            kp[64 * (h % 2):64 * (h % 2) + 64, h // 2, :] = (
                prompt[b, 0, :, gh, :].T.astype(BF16))
            vpa[:, h, 0:D] = prompt[b, 1, :, gh, :].astype(BF16)
        in_maps.append({
            "xqT": qT[b], "xkT": kT[b], "xvT": vT[b],
            "wqT": np.ascontiguousarray(Wq[cs, :].T).astype(BF16),
            "wkT": np.ascontiguousarray(Wk[cs, :].T).astype(BF16),
            "wvT": np.ascontiguousarray(Wv[cs, :].T).astype(BF16),
            "woT": np.ascontiguousarray(Wo[:, cs].T).astype(BF16),
            "bq": np.ascontiguousarray(bq[cs]).astype(f32).reshape(CL, 1),
            "bk": np.ascontiguousarray(bk[cs]).astype(f32).reshape(CL, 1),
            "bv": np.ascontiguousarray(bv[cs]).astype(f32).reshape(1, CL),
            "kpT": kp, "vp": vpa,
        })
    return in_maps


def _combine(results, bo):
    out = np.empty((B, S, E), np.float32)
    for b in range(B):
        acc = results[b * NG]["outT"].astype(np.float32)
        for g in range(1, NG):
            acc = acc + results[b * NG + g]["outT"]
        out[b] = acc.T
    if bo is not None and np.any(bo):
        out += np.asarray(bo, np.float32)
    return out


def run(inputs, trace=False):
    """Returns (output, exec_time_ns or None)."""
    from concourse import bass_utils

    nc = _get_nc()
    in_maps = _prep_in_maps(**{k: np.asarray(v) for k, v in inputs.items()})
    bo = np.asarray(inputs["bo"])
    res = bass_utils.run_bass_kernel_spmd(
        nc, in_maps, core_ids=list(range(NCORES)), trace=trace,
    )
    return _combine(res.results, bo), res.exec_time_ns


def kernel(**inputs):
    out, _ = run(inputs)
    return out



# revision 24
# speedup vs baseline: 1.1511x; 1.1511x over previous
"""Multi-head attention (B=2,S=4096,E=768,H=12,D=64 + 16-token K/V prompt
prefix) on 8 Trainium2 NeuronCores.

Sharding: 2 batches x 4 head-groups (3 heads each). Each core computes QKV
projections for its 3 heads, full attention over its batch, and a partial
output projection (its 192 ctx channels); the host sums the 4 partials per
batch.

v2 design (vs the 485us baseline, which was jointly PE- and ScalarE-bound):
  * scores matmuls run in fp8e4m3 with MatmulPerfMode.DoubleRow (d=64 split
    as [32 partitions x 2 interleave]); 0.5 cycles/row halves scores PE time.
    q/k live only in fp8; measured end-to-end rel-err impact ~1.2e-2.
  * ctx matmul is flipped: expt [k,128q] tiles are the *stationary* operand
    and v [k,65] the moving one, so each instruction streams 65 rows instead
    of 512 -- ctx PE time halves.  The ones-column in v still accumulates
    the softmax denominator (psc column 64).
  * exp is the 1/8-scaled softmax numerator; it is load-balanced across
    ScalarE (activation Exp, scale=1/8) AND Vector/Pool engines
    (tensor_tensor pow: expt = (e^{1/8})^s with a memset base tile).
  * ctx comes out of PSUM in [q, d] orientation; normalization is a single
    per-partition tensor_scalar divide; re-transposition to [d, q] for the
    out-projection rides the idle DMA engines via XBAR dma_start_transpose
    (two heads batched per transfer to satisfy the 128-col constraint).
  * PSUM: 2x[128,1024] scores + 1x[128,2,4,128pad] ctx + 2x[128,512]
    time-multiplexed (v-proj/bg q-proj/out-proj) = 8 banks exactly.
"""

import sys
import threading

import numpy as np

if "/opt/trn_rl_repo" not in sys.path:
    sys.path.insert(0, "/opt/trn_rl_repo")

import ml_dtypes

BF16 = ml_dtypes.bfloat16
FP8 = ml_dtypes.float8_e4m3

B, S, E, H, D, PP = 2, 4096, 768, 12, 64, 16
NCORES = 8
NG = 4          # head-groups (tensor parallel)
HL = H // NG    # 3 local heads
CL = HL * D     # 192 local channels
SKV = PP + S    # 4112
NKT = S // 128  # 32 full k-tiles (prefix handled separately)
QT = 1024       # q tile width for scores/exp/ctx
NSQ = S // QT   # 4
TRAIL = 12      # ctx matmuls trail scores by this many slots
NST = S // 128  # 32 v stiles
GAP = 6
# Schraudolph exp for the DVE share: bf16 bits of exp(s/8) ~=
# int16(s*SCHR_A + SCHR_B); one fused tensor_scalar (mult,add) writing
# through an int16 bitcast of the bf16 expt tile.  ~1.8% rms relative
# error on those tiles; the Act share stays exact, so total error scales
# with sqrt(phi).  C=7.5 centers the sawtooth; +0.5 makes trunc rounding.
SCHR_A = 128 * 1.4426950408889634 / 8   # 128*log2(e)/8
SCHR_B = 16256.5 - 7.5
# exp engine assignment pattern per slot: A=ScalarE (exact), D=Vector
# (Schraudolph).  GPSIMD cannot access PSUM; DVE has no transcendentals.
EXP_PAT = "ADADADADA"

_lock = threading.Lock()
_compiled = {}


def _build():
    import concourse.bass as bass  # noqa: F401
    import concourse.mybir as mybir
    import concourse.tile as tile
    from concourse import bacc

    f32 = mybir.dt.float32
    bf16 = mybir.dt.bfloat16
    fp8 = mybir.dt.float8e4
    i16 = mybir.dt.int16
    EXP = mybir.ActivationFunctionType.Exp
    IDN = mybir.ActivationFunctionType.Identity
    DIV = mybir.AluOpType.divide
    MUL = mybir.AluOpType.mult
    ADD = mybir.AluOpType.add
    DR = mybir.MatmulPerfMode.DoubleRow

    nc = bacc.Bacc("TRN2", target_bir_lowering=False, debug=False)

    xqT = nc.dram_tensor("xqT", [E, S], bf16, kind="ExternalInput").ap()
    xkT = nc.dram_tensor("xkT", [E, S], bf16, kind="ExternalInput").ap()
    xvT = nc.dram_tensor("xvT", [E, S], bf16, kind="ExternalInput").ap()
    wqT = nc.dram_tensor("wqT", [E, CL], bf16, kind="ExternalInput").ap()
    wkT = nc.dram_tensor("wkT", [E, CL], bf16, kind="ExternalInput").ap()
    wvT = nc.dram_tensor("wvT", [E, CL], bf16, kind="ExternalInput").ap()
    woT = nc.dram_tensor("woT", [CL, E], bf16, kind="ExternalInput").ap()
    bq = nc.dram_tensor("bq", [96, 2], f32, kind="ExternalInput").ap()
    bk = nc.dram_tensor("bk", [96, 2], f32, kind="ExternalInput").ap()
    bv = nc.dram_tensor("bv", [1, CL], f32, kind="ExternalInput").ap()
    kp8 = nc.dram_tensor("kp8", [96, 2, PP], fp8, kind="ExternalInput").ap()
    vp = nc.dram_tensor("vp", [PP, HL, D + 1], bf16, kind="ExternalInput").ap()
    outT = nc.dram_tensor("outT", [E, S], f32, kind="ExternalOutput").ap()

    with tile.TileContext(nc) as tc:
        with tc.tile_pool(name="persist", bufs=1) as pers:
            # q-projection weights/bias first: they gate the first matmuls
            wq_sb = pers.tile([128, 6, CL], bf16)
            nc.sync.dma_start(wq_sb[:], wqT.rearrange("(t p) c -> p t c", p=128))
            bq_sb = pers.tile([96, 2], f32)
            nc.sync.dma_start(bq_sb[:], bq[:])

            wk_sb = pers.tile([128, 6, CL], bf16)
            wv_sb = pers.tile([128, 6, CL], bf16)
            wo_sb = pers.tile([128, 2, E], bf16)
            bk_sb = pers.tile([96, 2], f32)
            bvb_sb = pers.tile([128, CL], f32)
            kp_sb = pers.tile([96, 2, PP], fp8)
            vp_sb = pers.tile([PP, HL, D + 1], bf16)

            # activations
            qT8 = pers.tile([96, 2, S], fp8)
            kT8 = pers.tile([96, 2, S], fp8)
            v_sb = pers.tile([128, NST, HL, D + 1], bf16)
            ctxT_sb = pers.tile([128, 2, S], bf16)
            expp_sb = pers.tile([PP, HL, S], bf16)  # prefix exp rows per head
            # normalized ctx staging, [q, d] orientation, manual sq-parity
            # double buffer; cn01 packs heads 0,1 so one XBAR dma transposes
            # 128 columns at once; cn2 pads head 2 with a junk half.
            cn01 = pers.tile([128, 2, 8, 2, D], bf16)
            cn2 = pers.tile([128, 2, 8, 2, D], bf16)

            nc.vector.memset(v_sb[:, :, :, D:D + 1], 1.0)
            nc.vector.memset(cn2[:], 0.0)

            # ---------------- Phase 1: Q(sq0) / K projections ----------------
            with (
                tc.tile_pool(name="ps_prlg", bufs=2, space="PSUM") as pprlg,
                tc.tile_pool(name="xq_pool", bufs=8) as xq_pool,
            ):
                def proj_block(xin, wsb, bsb, dst8, sq):
                    xts = []
                    for ech in range(6):
                        xt = xq_pool.tile([128, QT], bf16, tag="xt", name="xt")
                        nc.sync.dma_start(
                            xt[:],
                            xin[ech * 128:(ech + 1) * 128,
                                sq * QT:(sq + 1) * QT],
                        )
                        xts.append(xt)
                    for i in range(2):
                        for n in range(2):
                            p = pprlg.tile([128, 512], f32, tag="pp", name="pp")
                            ns = slice(n * 512, (n + 1) * 512)
                            for ech in range(6):
                                nc.tensor.matmul(
                                    p[0:96, :],
                                    wsb[:, ech, i * 96:(i + 1) * 96],
                                    xts[ech][:, ns],
                                    start=(ech == 0), stop=(ech == 5),
                                )
                            qs = slice(sq * QT + n * 512, sq * QT + (n + 1) * 512)
                            # evac on ScalarE (Copy + per-partition bias) to
                            # keep DVE free for exp work
                            nc.scalar.activation(
                                dst8[0:96, i, qs], p[0:96, :], IDN,
                                bias=bsb[:, i:i + 1])

                proj_block(xqT, wq_sb, bq_sb, qT8, 0)
                # stream in the remaining weights behind the critical q DMAs
                nc.sync.dma_start(
                    wk_sb[:], wkT.rearrange("(t p) c -> p t c", p=128))
                nc.sync.dma_start(bk_sb[:], bk[:])
                nc.sync.dma_start(kp_sb[:], kp8[:])
                nc.sync.dma_start(
                    wv_sb[:], wvT.rearrange("(t p) c -> p t c", p=128))
                nc.sync.dma_start(bvb_sb[:], bv.to_broadcast((128, CL)))
                nc.sync.dma_start(vp_sb[:], vp[:])
                nc.sync.dma_start(wo_sb[:, 0, :], woT[0:128, :])
                nc.sync.dma_start(wo_sb[0:64, 1, :], woT[128:CL, :])

                # sq0 prompt-prefix scores+exp: ScalarE starts early
                for h in range(HL):
                    hp = slice(32 * h, 32 * h + 32)
                    for n in range(2):
                        pf = pprlg.tile([128, 512], f32, tag="pp", name="pf")
                        qs = slice(n * 512, (n + 1) * 512)
                        nc.tensor.matmul(
                            pf[0:PP, :], kp_sb[hp, :, :], qT8[hp, :, qs],
                            start=True, stop=True, perf_mode=DR,
                        )
                        nc.scalar.activation(
                            expp_sb[:, h, qs], pf[0:PP, :], EXP, scale=0.125)

                for sq in range(NSQ):
                    proj_block(xkT, wk_sb, bk_sb, kT8, sq)

            # ---------- attention + V-proj + out-proj: one slot stream ----------
            with (
                tc.tile_pool(name="ps_s", bufs=2, space="PSUM") as ps_s,
                tc.tile_pool(name="ps_c", bufs=1, space="PSUM") as ps_c,
                tc.tile_pool(name="ps_sm", bufs=2, space="PSUM") as ps_sm,
                tc.tile_pool(name="expt_pool", bufs=20) as expt_pool,
                tc.tile_pool(name="xv_pool", bufs=8) as xv_pool,
                tc.tile_pool(name="xq2_pool", bufs=7) as xq2_pool,
                tc.tile_pool(name="out_pool", bufs=4) as out_pool,
                tc.tile_pool(name="nrm_pool", bufs=2) as nrm_pool,
            ):
                expcnt = [0]

                def emit_exp(dst, src, rows, exact=False):
                    eng = "A" if exact else EXP_PAT[expcnt[0] % len(EXP_PAT)]
                    expcnt[0] += 1
                    if eng == "A":
                        nc.scalar.activation(dst, src, EXP, scale=0.125)
                    else:
                        nc.vector.tensor_scalar(
                            dst.bitcast(i16), src, float(SCHR_A),
                            float(SCHR_B), MUL, ADD)

                def emit_prefix(sq, h):
                    hp = slice(32 * h, 32 * h + 32)
                    psp = ps_s.tile([128, QT], f32, tag="pss", name="psp")
                    for n in range(2):
                        ns = slice(n * 512, (n + 1) * 512)
                        qs = slice(sq * QT + n * 512, sq * QT + (n + 1) * 512)
                        nc.tensor.matmul(
                            psp[0:PP, ns], kp_sb[hp, :, :], qT8[hp, :, qs],
                            start=True, stop=True, perf_mode=DR,
                        )
                    emit_exp(expp_sb[:, h, sq * QT:(sq + 1) * QT],
                             psp[0:PP, :], PP, exact=True)

                # Background q-projection for sq 1..3 (drained through the
                # time-multiplexed sm pool, a few ops per designated slot)
                def make_bg_qproj(sq):
                    ops = []
                    state = {}

                    def dma_op():
                        tiles = []
                        for ech in range(6):
                            xt2 = xq2_pool.tile([128, QT], bf16, tag="xt2",
                                                name="xt2")
                            nc.sync.dma_start(
                                xt2[:],
                                xqT[ech * 128:(ech + 1) * 128,
                                    sq * QT:(sq + 1) * QT],
                            )
                            tiles.append(xt2)
                        state["xt"] = tiles

                    ops.append(dma_op)

                    def mk_group(i, n):
                        def op():
                            p = ps_sm.tile([128, 512], f32, tag="sm",
                                           name="pq")
                            ns = slice(n * 512, (n + 1) * 512)
                            for ech in range(6):
                                nc.tensor.matmul(
                                    p[0:96, :],
                                    wq_sb[:, ech, i * 96:(i + 1) * 96],
                                    state["xt"][ech][:, ns],
                                    start=(ech == 0), stop=(ech == 5),
                                )
                            qs = slice(sq * QT + n * 512,
                                       sq * QT + (n + 1) * 512)
                            nc.scalar.activation(
                                qT8[0:96, i, qs], p[0:96, :], IDN,
                                bias=bq_sb[:, i:i + 1])
                        return op

                    for i in range(2):
                        for n in range(2):
                            ops.append(mk_group(i, n))
                    for h in range(HL):
                        ops.append(lambda h=h: emit_prefix(sq, h))
                    return ops

                bg_work = []
                for nb, sqb in ((16, 1), (108, 2), (204, 3)):
                    for k, op in enumerate(make_bg_qproj(sqb)):
                        bg_work.append((nb + 5 * k, op))

                # xv DMA loads, one sq-group of 6 chunks at a time
                xvts = {}

                def load_xv(sqx):
                    tiles = []
                    for ech in range(6):
                        xvt = xv_pool.tile([128, QT], bf16, tag="xvt",
                                           name="xvt")
                        nc.sync.dma_start(
                            xvt[:],
                            xvT[ech * 128:(ech + 1) * 128,
                                sqx * QT:(sqx + 1) * QT],
                        )
                        tiles.append(xvt)
                    xvts[sqx] = tiles

                def emit_vproj(st):
                    sqx, stl = st // (QT // 128), st % (QT // 128)
                    if st == 0:
                        load_xv(0)
                    if stl == 0 and sqx + 1 < NSQ:
                        load_xv(sqx + 1)
                    pv = ps_sm.tile([128, 512], f32, tag="sm", name="pv")
                    for ech in range(6):
                        nc.tensor.matmul(
                            pv[:, 0:CL],
                            xvts[sqx][ech][:, stl * 128:(stl + 1) * 128],
                            wv_sb[:, ech, :],
                            start=(ech == 0), stop=(ech == 5),
                        )
                    nc.vector.tensor_add(
                        v_sb[:, st, :, 0:D],
                        pv[:, 0:CL].rearrange("p (h d) -> p h d", h=HL),
                        bvb_sb[:].rearrange("p (h d) -> p h d", h=HL),
                    )
                    if stl == (QT // 128) - 1:
                        del xvts[sqx]

                def emit_scores_exp(sq, h, kt):
                    hp = slice(32 * h, 32 * h + 32)
                    pss = ps_s.tile([128, QT], f32, tag="pss", name="pss")
                    expt = expt_pool.tile([128, QT], bf16, tag="expt",
                                          name="expt")
                    for n in range(2):
                        ns = slice(n * 512, (n + 1) * 512)
                        qs = slice(sq * QT + n * 512, sq * QT + (n + 1) * 512)
                        nc.tensor.matmul(
                            pss[:, ns],
                            kT8[hp, :, kt * 128:(kt + 1) * 128],
                            qT8[hp, :, qs],
                            start=True, stop=True, perf_mode=DR,
                        )
                    emit_exp(expt[:], pss[:], 128)
                    return expt

                psc_tiles = {}
                outproj_work = []

                def emit_ctx(sq, h, kt, expt):
                    key = (sq, h)
                    if kt == 0:
                        psc_tiles[key] = ps_c.tile(
                            [128, 2, 4, 128], f32, tag="psc", name="psc")
                    psc = psc_tiles[key]
                    # PSUM zero-region (2KB bank) semantics: only the first
                    # slice per bank may carry start=True (it marks the whole
                    # region pending-zero; sibling slices' first writes then
                    # overwrite-on-first-touch), and only the last slice may
                    # carry stop=True (it clears the whole region's group).
                    for qb in range(8):
                        nc.tensor.matmul(
                            psc[:, qb // 4, qb % 4, 0:D + 1],
                            expt[:, qb * 128:(qb + 1) * 128],
                            v_sb[:, kt, h, :],
                            start=(kt == 0 and qb % 4 == 0),
                            stop=(kt == NKT - 1 and qb % 4 == 3),
                        )
                    if kt == TRAIL - 1:
                        # prompt-prefix ctx contribution (reads expp_sb rows)
                        for qb in range(8):
                            qs = slice(sq * QT + qb * 128,
                                       sq * QT + (qb + 1) * 128)
                            nc.tensor.matmul(
                                psc[:, qb // 4, qb % 4, 0:D + 1],
                                expp_sb[:, h, qs],
                                vp_sb[:, h, :],
                                start=False, stop=False,
                            )
                    if kt == NKT - 1:
                        emit_norm(sq, h, psc)
                        del psc_tiles[key]

                def emit_norm(sq, h, psc):
                    par = sq % 2
                    cn = cn2 if h == 2 else cn01
                    hh = 0 if h == 2 else h
                    # hw tensor_scalar has no divide: batched reciprocal of
                    # the 8 denominator columns, then per-block multiplies
                    rc = nrm_pool.tile([128, 8], f32, tag="rc", name="rc")
                    nc.vector.reciprocal(
                        rc[:].rearrange("p (a b) -> p a b", a=2),
                        psc[:, :, :, D:D + 1].squeeze(3))
                    for qb in range(8):
                        nc.vector.tensor_scalar(
                            cn[:, par, qb, hh, :],
                            psc[:, qb // 4, qb % 4, 0:D],
                            rc[:, qb:qb + 1],
                            None, MUL,
                        )
                    if h >= 1:
                        # heads 0,1 pair (after h1) / head 2 -> XBAR transpose
                        cnin, pr = (cn01, 0) if h == 1 else (cn2, 1)
                        for qb in range(8):
                            qs = slice(sq * QT + qb * 128,
                                       sq * QT + (qb + 1) * 128)
                            nc.sync.dma_start_transpose(
                                ctxT_sb[:, pr, qs], cnin[:, par, qb, :, :])
                    if h == HL - 1:
                        for et in range(6):
                            for qn in range(2):
                                outproj_work.append((et, sq * 2 + qn))

                def emit_outproj_tile(et, qn):
                    es = slice(et * 128, (et + 1) * 128)
                    qs = slice(qn * 512, (qn + 1) * 512)
                    po3 = ps_sm.tile([128, 512], f32, tag="sm", name="po3")
                    nc.tensor.matmul(
                        po3[:], wo_sb[:, 0, es], ctxT_sb[:, 0, qs],
                        start=True, stop=False,
                    )
                    nc.tensor.matmul(
                        po3[:], wo_sb[0:64, 1, es], ctxT_sb[0:64, 1, qs],
                        start=False, stop=True,
                    )
                    # DMA cannot read PSUM: stage through SBUF via ScalarE
                    ot = out_pool.tile([128, 512], f32, tag="ot", name="ot")
                    nc.scalar.activation(ot[:], po3[:], IDN)
                    nc.sync.dma_start(outT[es, qs], ot[:])

                slots = [(sq, h, kt)
                         for sq in range(NSQ)
                         for h in range(HL)
                         for kt in range(NKT)]
                pending = []

                def pop_one():
                    (s2, e2) = pending.pop(0)
                    emit_ctx(*s2, e2)

                vst = 0
                for j, slot in enumerate(slots):
                    expt = emit_scores_exp(*slot)
                    pending.append((slot, expt))
                    if vst < NST:
                        emit_vproj(vst)
                        vst += 1
                    trail_eff = TRAIL if j < len(slots) - 34 else 2
                    for _ in range(3):
                        if not pending:
                            break
                        need = (trail_eff + GAP if pending[0][0][2] == 0
                                else trail_eff)
                        if len(pending) > need:
                            pop_one()
                        else:
                            break
                    if bg_work and j >= bg_work[0][0]:
                        bg_work.pop(0)[1]()
                    elif outproj_work and j % 2 == 0:
                        # every other slot: the PSUM->DRAM store's read
                        # completion is what frees the sm bank (~2-3us)
                        emit_outproj_tile(*outproj_work.pop(0))
                while pending:
                    pop_one()
                    if outproj_work:
                        emit_outproj_tile(*outproj_work.pop(0))
                for _, op in bg_work:
                    op()
                while outproj_work:
                    emit_outproj_tile(*outproj_work.pop(0))

    nc.compile()
    return nc


def _get_nc():
    with _lock:
        if "nc" not in _compiled:
            _compiled["nc"] = _build()
        return _compiled["nc"]


def _chan_perm():
    # fp8 DoubleRow layout: channel (p, i) <- head p//32, d = i*32 + p%32
    cols = np.empty((2, 96), np.int64)
    for i in range(2):
        for p in range(96):
            cols[i, p] = (p // 32) * 64 + i * 32 + (p % 32)
    return cols.reshape(-1)  # j = i*96 + p


def _prep_in_maps(query, key, value, prompt, Wq, bq, Wk, bk, Wv, bv, Wo, bo):
    f32 = np.float32
    qT = [np.ascontiguousarray(query[b].T).astype(BF16) for b in range(B)]
    kT = [np.ascontiguousarray(key[b].T).astype(BF16) for b in range(B)]
    vT = [np.ascontiguousarray(value[b].T).astype(BF16) for b in range(B)]
    perm = _chan_perm()
    in_maps = []
    for core in range(NCORES):
        b, g = core // NG, core % NG
        cs = slice(g * CL, (g + 1) * CL)
        Wq_g = np.asarray(Wq)[cs, :]
        Wk_g = np.asarray(Wk)[cs, :]
        bq_g = np.asarray(bq)[cs].astype(f32)
        bk_g = np.asarray(bk)[cs].astype(f32)
        kp = np.zeros((96, 2, PP), FP8)
        for i in range(2):
            for p in range(96):
                gh = g * HL + p // 32
                d = i * 32 + p % 32
                kp[p, i, :] = prompt[b, 0, :, gh, d].astype(FP8)
        vpa = np.zeros((PP, HL, D + 1), BF16)
        vpa[:, :, D] = 1.0
        for h in range(HL):
            gh = g * HL + h
            vpa[:, h, 0:D] = prompt[b, 1, :, gh, :].astype(BF16)
        in_maps.append({
            "xqT": qT[b], "xkT": kT[b], "xvT": vT[b],
            "wqT": np.ascontiguousarray(Wq_g[perm, :].T).astype(BF16),
            "wkT": np.ascontiguousarray(Wk_g[perm, :].T).astype(BF16),
            "wvT": np.ascontiguousarray(np.asarray(Wv)[cs, :].T).astype(BF16),
            "woT": np.ascontiguousarray(np.asarray(Wo)[:, cs].T).astype(BF16),
            "bq": np.ascontiguousarray(
                bq_g[perm].reshape(2, 96).T).astype(f32),
            "bk": np.ascontiguousarray(
                bk_g[perm].reshape(2, 96).T).astype(f32),
            "bv": np.ascontiguousarray(
                np.asarray(bv)[cs]).astype(f32).reshape(1, CL),
            "kp8": kp, "vp": vpa,
        })
    return in_maps


def _combine(results, bo):
    out = np.empty((B, S, E), np.float32)
    for b in range(B):
        acc = results[b * NG]["outT"].astype(np.float32)
        for g in range(1, NG):
            acc = acc + results[b * NG + g]["outT"]
        out[b] = acc.T
    if bo is not None and np.any(bo):
        out += np.asarray(bo, np.float32)
    return out


def run(inputs, trace=False):
    """Returns (output, exec_time_ns or None)."""
    from concourse import bass_utils

    nc = _get_nc()
    in_maps = _prep_in_maps(**{k: np.asarray(v) for k, v in inputs.items()})
    bo = np.asarray(inputs["bo"])
    res = bass_utils.run_bass_kernel_spmd(
        nc, in_maps, core_ids=list(range(NCORES)), trace=trace,
    )
    return _combine(res.results, bo), res.exec_time_ns


def kernel(**inputs):
    out, _ = run(inputs)
    return out


# revision 28
# speedup vs baseline: 1.1965x; 1.0395x over previous
"""Multi-head attention (B=2,S=4096,E=768,H=12,D=64 + 16-token K/V prompt
prefix) on 8 Trainium2 NeuronCores.

Sharding: 2 batches x 4 head-groups (3 heads each). Each core computes QKV
projections for its 3 heads, full attention over its batch, and a partial
output projection (its 192 ctx channels); the host sums the 4 partials per
batch.

v2 design (vs the 485us baseline, which was jointly PE- and ScalarE-bound):
  * scores matmuls run in fp8e4m3 with MatmulPerfMode.DoubleRow (d=64 split
    as [32 partitions x 2 interleave]); 0.5 cycles/row halves scores PE time.
    q/k live only in fp8; measured end-to-end rel-err impact ~1.2e-2.
  * ctx matmul is flipped: expt [k,128q] tiles are the *stationary* operand
    and v [k,65] the moving one, so each instruction streams 65 rows instead
    of 512 -- ctx PE time halves.  The ones-column in v still accumulates
    the softmax denominator (psc column 64).
  * exp is the 1/8-scaled softmax numerator; it is load-balanced across
    ScalarE (activation Exp, scale=1/8) AND Vector/Pool engines
    (tensor_tensor pow: expt = (e^{1/8})^s with a memset base tile).
  * ctx comes out of PSUM in [q, d] orientation; normalization is a single
    per-partition tensor_scalar divide; re-transposition to [d, q] for the
    out-projection rides the idle DMA engines via XBAR dma_start_transpose
    (two heads batched per transfer to satisfy the 128-col constraint).
  * PSUM: 2x[128,1024] scores + 1x[128,2,4,128pad] ctx + 2x[128,512]
    time-multiplexed (v-proj/bg q-proj/out-proj) = 8 banks exactly.
"""

import sys
import threading

import numpy as np

if "/opt/trn_rl_repo" not in sys.path:
    sys.path.insert(0, "/opt/trn_rl_repo")

import ml_dtypes

BF16 = ml_dtypes.bfloat16
FP8 = ml_dtypes.float8_e4m3

B, S, E, H, D, PP = 2, 4096, 768, 12, 64, 16
NCORES = 8
NG = 4          # head-groups (tensor parallel)
HL = H // NG    # 3 local heads
CL = HL * D     # 192 local channels
SKV = PP + S    # 4112
NKT = S // 128  # 32 full k-tiles (prefix handled separately)
QT = 1024       # q tile width for scores/exp/ctx
NSQ = S // QT   # 4
TRAIL = 12      # ctx matmuls trail scores by this many slots
NST = S // 128  # 32 v stiles
GAP = 6
# Schraudolph exp for the DVE share: bf16 bits of exp(s/8) ~=
# int16(s*SCHR_A + SCHR_B); one fused tensor_scalar (mult,add) writing
# through an int16 bitcast of the bf16 expt tile.  ~1.8% rms relative
# error on those tiles; the Act share stays exact, so total error scales
# with sqrt(phi).  C=7.5 centers the sawtooth; +0.5 makes trunc rounding.
SCHR_A = 128 * 1.4426950408889634 / 8   # 128*log2(e)/8
SCHR_B = 16256.5 - 7.5
# exp engine assignment pattern per slot: A=ScalarE (exact), D=Vector
# (Schraudolph).  GPSIMD cannot access PSUM; DVE has no transcendentals.
EXP_PAT = "ADADADADA"

_lock = threading.Lock()
_compiled = {}


def _build():
    import concourse.bass as bass  # noqa: F401
    import concourse.mybir as mybir
    import concourse.tile as tile
    from concourse import bacc

    f32 = mybir.dt.float32
    bf16 = mybir.dt.bfloat16
    fp8 = mybir.dt.float8e4
    i16 = mybir.dt.int16
    EXP = mybir.ActivationFunctionType.Exp
    IDN = mybir.ActivationFunctionType.Identity
    DIV = mybir.AluOpType.divide
    MUL = mybir.AluOpType.mult
    ADD = mybir.AluOpType.add
    DR = mybir.MatmulPerfMode.DoubleRow

    nc = bacc.Bacc("TRN2", target_bir_lowering=False, debug=False)

    xqT = nc.dram_tensor("xqT", [E, S], bf16, kind="ExternalInput").ap()
    xkT = nc.dram_tensor("xkT", [E, S], bf16, kind="ExternalInput").ap()
    xvT = nc.dram_tensor("xvT", [E, S], bf16, kind="ExternalInput").ap()
    wqT = nc.dram_tensor("wqT", [E, CL], bf16, kind="ExternalInput").ap()
    wkT = nc.dram_tensor("wkT", [E, CL], bf16, kind="ExternalInput").ap()
    wvT = nc.dram_tensor("wvT", [E, CL], bf16, kind="ExternalInput").ap()
    woT = nc.dram_tensor("woT", [CL, E], bf16, kind="ExternalInput").ap()
    bq = nc.dram_tensor("bq", [96, 2], f32, kind="ExternalInput").ap()
    bk = nc.dram_tensor("bk", [96, 2], f32, kind="ExternalInput").ap()
    bv = nc.dram_tensor("bv", [1, CL], f32, kind="ExternalInput").ap()
    kp8 = nc.dram_tensor("kp8", [96, 2, PP], fp8, kind="ExternalInput").ap()
    vp = nc.dram_tensor("vp", [PP, HL, D + 1], bf16, kind="ExternalInput").ap()
    outT = nc.dram_tensor("outT", [E, S], f32, kind="ExternalOutput").ap()

    with tile.TileContext(nc) as tc:
        with tc.tile_pool(name="persist", bufs=1) as pers:
            # q-projection weights/bias first: they gate the first matmuls
            wq_sb = pers.tile([128, 6, CL], bf16)
            nc.sync.dma_start(wq_sb[:], wqT.rearrange("(t p) c -> p t c", p=128))
            bq_sb = pers.tile([96, 2], f32)
            nc.sync.dma_start(bq_sb[:], bq[:])

            wk_sb = pers.tile([128, 6, CL], bf16)
            wv_sb = pers.tile([128, 6, CL], bf16)
            wo_sb = pers.tile([128, 2, E], bf16)
            bk_sb = pers.tile([96, 2], f32)
            bvb_sb = pers.tile([128, CL], f32)
            kp_sb = pers.tile([96, 2, PP], fp8)
            vp_sb = pers.tile([PP, HL, D + 1], bf16)

            # activations
            qT8 = pers.tile([96, 2, S], fp8)
            kT8 = pers.tile([96, 2, S], fp8)
            v_sb = pers.tile([128, NST, HL, D + 1], bf16)
            ctxT_sb = pers.tile([128, 2, S], bf16)
            expp_sb = pers.tile([PP, HL, S], bf16)  # prefix exp rows per head
            # normalized ctx staging, [q, d] orientation, manual sq-parity
            # double buffer; cn01 packs heads 0,1 so one XBAR dma transposes
            # 128 columns at once; cn2 pads head 2 with a junk half.
            cn01 = pers.tile([128, 2, 8, 2, D], bf16)
            cn2 = pers.tile([128, 2, 8, 2, D], bf16)

            nc.vector.memset(v_sb[:, :, :, D:D + 1], 1.0)
            nc.vector.memset(cn2[:], 0.0)

            # ---------------- Phase 1: Q(sq0) / K projections ----------------
            with (
                tc.tile_pool(name="ps_prlg", bufs=2, space="PSUM") as pprlg,
                tc.tile_pool(name="xq_pool", bufs=8) as xq_pool,
            ):
                def proj_block(xin, wsb, bsb, dst8, sq):
                    xts = []
                    for ech in range(6):
                        xt = xq_pool.tile([128, QT], bf16, tag="xt", name="xt")
                        nc.sync.dma_start(
                            xt[:],
                            xin[ech * 128:(ech + 1) * 128,
                                sq * QT:(sq + 1) * QT],
                        )
                        xts.append(xt)
                    for i in range(2):
                        for n in range(2):
                            p = pprlg.tile([128, 512], f32, tag="pp", name="pp")
                            ns = slice(n * 512, (n + 1) * 512)
                            for ech in range(6):
                                nc.tensor.matmul(
                                    p[0:96, :],
                                    wsb[:, ech, i * 96:(i + 1) * 96],
                                    xts[ech][:, ns],
                                    start=(ech == 0), stop=(ech == 5),
                                )
                            qs = slice(sq * QT + n * 512, sq * QT + (n + 1) * 512)
                            # evac on ScalarE (Copy + per-partition bias) to
                            # keep DVE free for exp work
                            nc.scalar.activation(
                                dst8[0:96, i, qs], p[0:96, :], IDN,
                                bias=bsb[:, i:i + 1])

                proj_block(xqT, wq_sb, bq_sb, qT8, 0)
                # stream in the remaining weights behind the critical q DMAs
                nc.sync.dma_start(
                    wk_sb[:], wkT.rearrange("(t p) c -> p t c", p=128))
                nc.sync.dma_start(bk_sb[:], bk[:])
                nc.sync.dma_start(kp_sb[:], kp8[:])
                nc.sync.dma_start(
                    wv_sb[:], wvT.rearrange("(t p) c -> p t c", p=128))
                nc.sync.dma_start(bvb_sb[:], bv.to_broadcast((128, CL)))
                nc.sync.dma_start(vp_sb[:], vp[:])
                nc.sync.dma_start(wo_sb[:, 0, :], woT[0:128, :])
                nc.sync.dma_start(wo_sb[0:64, 1, :], woT[128:CL, :])

                # sq0 prompt-prefix scores+exp: ScalarE starts early
                for h in range(HL):
                    hp = slice(32 * h, 32 * h + 32)
                    for n in range(2):
                        pf = pprlg.tile([128, 512], f32, tag="pp", name="pf")
                        qs = slice(n * 512, (n + 1) * 512)
                        nc.tensor.matmul(
                            pf[0:PP, :], kp_sb[hp, :, :], qT8[hp, :, qs],
                            start=True, stop=True, perf_mode=DR,
                        )
                        nc.scalar.activation(
                            expp_sb[:, h, qs], pf[0:PP, :], EXP, scale=0.125)

                for sq in range(NSQ):
                    proj_block(xkT, wk_sb, bk_sb, kT8, sq)

            # ---------- attention + V-proj + out-proj: one slot stream ----------
            with (
                tc.tile_pool(name="ps_s", bufs=4, space="PSUM") as ps_s,
                tc.tile_pool(name="ps_c", bufs=1, space="PSUM") as ps_c,
                tc.tile_pool(name="ps_sm", bufs=2, space="PSUM") as ps_sm,
                tc.tile_pool(name="expt_pool", bufs=20) as expt_pool,
                tc.tile_pool(name="xv_pool", bufs=8) as xv_pool,
                tc.tile_pool(name="xq2_pool", bufs=7) as xq2_pool,
                tc.tile_pool(name="out_pool", bufs=4) as out_pool,
                tc.tile_pool(name="nrm_pool", bufs=2) as nrm_pool,
            ):
                expcnt = [0]

                def emit_exp(dst, src, rows, exact=False):
                    eng = "A" if exact else EXP_PAT[expcnt[0] % len(EXP_PAT)]
                    expcnt[0] += 1
                    if eng == "A":
                        nc.scalar.activation(dst, src, EXP, scale=0.125)
                    else:
                        nc.vector.tensor_scalar(
                            dst.bitcast(i16), src, float(SCHR_A),
                            float(SCHR_B), MUL, ADD)

                def emit_prefix(sq, h):
                    hp = slice(32 * h, 32 * h + 32)
                    for n in range(2):
                        psp = ps_s.tile([128, 512], f32, tag="pss", name="psp")
                        qs = slice(sq * QT + n * 512, sq * QT + (n + 1) * 512)
                        nc.tensor.matmul(
                            psp[0:PP, :], kp_sb[hp, :, :], qT8[hp, :, qs],
                            start=True, stop=True, perf_mode=DR,
                        )
                        emit_exp(expp_sb[:, h, qs], psp[0:PP, :], PP,
                                 exact=True)

                # Background q-projection for sq 1..3 (drained through the
                # time-multiplexed sm pool, a few ops per designated slot)
                def make_bg_qproj(sq):
                    ops = []
                    state = {}

                    def dma_op():
                        tiles = []
                        for ech in range(6):
                            xt2 = xq2_pool.tile([128, QT], bf16, tag="xt2",
                                                name="xt2")
                            nc.sync.dma_start(
                                xt2[:],
                                xqT[ech * 128:(ech + 1) * 128,
                                    sq * QT:(sq + 1) * QT],
                            )
                            tiles.append(xt2)
                        state["xt"] = tiles

                    ops.append(dma_op)

                    def mk_group(i, n):
                        def op():
                            p = ps_sm.tile([128, 512], f32, tag="sm",
                                           name="pq")
                            ns = slice(n * 512, (n + 1) * 512)
                            for ech in range(6):
                                nc.tensor.matmul(
                                    p[0:96, :],
                                    wq_sb[:, ech, i * 96:(i + 1) * 96],
                                    state["xt"][ech][:, ns],
                                    start=(ech == 0), stop=(ech == 5),
                                )
                            qs = slice(sq * QT + n * 512,
                                       sq * QT + (n + 1) * 512)
                            nc.scalar.activation(
                                qT8[0:96, i, qs], p[0:96, :], IDN,
                                bias=bq_sb[:, i:i + 1])
                        return op

                    for i in range(2):
                        for n in range(2):
                            ops.append(mk_group(i, n))
                    for h in range(HL):
                        ops.append(lambda h=h: emit_prefix(sq, h))
                    return ops

                bg_work = []
                for nb, sqb in ((16, 1), (108, 2), (204, 3)):
                    for k, op in enumerate(make_bg_qproj(sqb)):
                        bg_work.append((nb + 5 * k, op))

                # xv DMA loads, one sq-group of 6 chunks at a time
                xvts = {}

                def load_xv(sqx):
                    tiles = []
                    for ech in range(6):
                        xvt = xv_pool.tile([128, QT], bf16, tag="xvt",
                                           name="xvt")
                        nc.sync.dma_start(
                            xvt[:],
                            xvT[ech * 128:(ech + 1) * 128,
                                sqx * QT:(sqx + 1) * QT],
                        )
                        tiles.append(xvt)
                    xvts[sqx] = tiles

                def emit_vproj(st):
                    sqx, stl = st // (QT // 128), st % (QT // 128)
                    if st == 0:
                        load_xv(0)
                    if stl == 0 and sqx + 1 < NSQ:
                        load_xv(sqx + 1)
                    pv = ps_sm.tile([128, 512], f32, tag="sm", name="pv")
                    for ech in range(6):
                        nc.tensor.matmul(
                            pv[:, 0:CL],
                            xvts[sqx][ech][:, stl * 128:(stl + 1) * 128],
                            wv_sb[:, ech, :],
                            start=(ech == 0), stop=(ech == 5),
                        )
                    nc.vector.tensor_add(
                        v_sb[:, st, :, 0:D],
                        pv[:, 0:CL].rearrange("p (h d) -> p h d", h=HL),
                        bvb_sb[:].rearrange("p (h d) -> p h d", h=HL),
                    )
                    if stl == (QT // 128) - 1:
                        del xvts[sqx]

                def emit_scores_exp(sq, h, kt):
                    hp = slice(32 * h, 32 * h + 32)
                    expt = expt_pool.tile([128, QT], bf16, tag="expt",
                                          name="expt")
                    # one single-bank pss tile (4-deep rotation) per 512-q
                    # half: keeps 2 slot-generations in flight so scores
                    # never wait on the exp of the previous-but-one slot
                    for n in range(2):
                        pss = ps_s.tile([128, 512], f32, tag="pss",
                                        name="pss")
                        qs = slice(sq * QT + n * 512, sq * QT + (n + 1) * 512)
                        nc.tensor.matmul(
                            pss[:],
                            kT8[hp, :, kt * 128:(kt + 1) * 128],
                            qT8[hp, :, qs],
                            start=True, stop=True, perf_mode=DR,
                        )
                        emit_exp(expt[:, n * 512:(n + 1) * 512], pss[:], 128)
                    return expt

                psc_tiles = {}
                outproj_work = []

                def emit_ctx(sq, h, kt, expt):
                    key = (sq, h)
                    if kt == 0:
                        psc_tiles[key] = ps_c.tile(
                            [128, 2, 4, 128], f32, tag="psc", name="psc")
                    psc = psc_tiles[key]
                    # PSUM zero-region (2KB bank) semantics: only the first
                    # slice per bank may carry start=True (it marks the whole
                    # region pending-zero; sibling slices' first writes then
                    # overwrite-on-first-touch), and only the last slice may
                    # carry stop=True (it clears the whole region's group).
                    for qb in range(8):
                        nc.tensor.matmul(
                            psc[:, qb // 4, qb % 4, 0:D + 1],
                            expt[:, qb * 128:(qb + 1) * 128],
                            v_sb[:, kt, h, :],
                            start=(kt == 0 and qb % 4 == 0),
                            stop=(kt == NKT - 1 and qb % 4 == 3),
                        )
                    if kt == TRAIL - 1:
                        # prompt-prefix ctx contribution (reads expp_sb rows)
                        for qb in range(8):
                            qs = slice(sq * QT + qb * 128,
                                       sq * QT + (qb + 1) * 128)
                            nc.tensor.matmul(
                                psc[:, qb // 4, qb % 4, 0:D + 1],
                                expp_sb[:, h, qs],
                                vp_sb[:, h, :],
                                start=False, stop=False,
                            )
                    if kt == NKT - 1:
                        emit_norm(sq, h, psc)
                        del psc_tiles[key]

                def emit_norm(sq, h, psc):
                    par = sq % 2
                    cn = cn2 if h == 2 else cn01
                    hh = 0 if h == 2 else h
                    # hw tensor_scalar has no divide: batched reciprocal of
                    # the 8 denominator columns, then per-block multiplies
                    rc = nrm_pool.tile([128, 8], f32, tag="rc", name="rc")
                    nc.vector.reciprocal(
                        rc[:].rearrange("p (a b) -> p a b", a=2),
                        psc[:, :, :, D:D + 1].squeeze(3))
                    for qb in range(8):
                        nc.vector.tensor_scalar(
                            cn[:, par, qb, hh, :],
                            psc[:, qb // 4, qb % 4, 0:D],
                            rc[:, qb:qb + 1],
                            None, MUL,
                        )
                    if h >= 1:
                        # heads 0,1 pair (after h1) / head 2 -> XBAR transpose
                        cnin, pr = (cn01, 0) if h == 1 else (cn2, 1)
                        for qb in range(8):
                            qs = slice(sq * QT + qb * 128,
                                       sq * QT + (qb + 1) * 128)
                            nc.sync.dma_start_transpose(
                                ctxT_sb[:, pr, qs], cnin[:, par, qb, :, :])
                    if h == HL - 1:
                        for et in range(6):
                            for qn in range(2):
                                outproj_work.append((et, sq * 2 + qn))

                def emit_outproj_tile(et, qn):
                    es = slice(et * 128, (et + 1) * 128)
                    qs = slice(qn * 512, (qn + 1) * 512)
                    po3 = ps_sm.tile([128, 512], f32, tag="sm", name="po3")
                    nc.tensor.matmul(
                        po3[:], wo_sb[:, 0, es], ctxT_sb[:, 0, qs],
                        start=True, stop=False,
                    )
                    nc.tensor.matmul(
                        po3[:], wo_sb[0:64, 1, es], ctxT_sb[0:64, 1, qs],
                        start=False, stop=True,
                    )
                    # DMA cannot read PSUM: stage through SBUF via DVE
                    ot = out_pool.tile([128, 512], f32, tag="ot", name="ot")
                    nc.vector.tensor_copy(ot[:], po3[:])
                    nc.sync.dma_start(outT[es, qs], ot[:])

                slots = [(sq, h, kt)
                         for sq in range(NSQ)
                         for h in range(HL)
                         for kt in range(NKT)]
                pending = []

                def pop_one():
                    (s2, e2) = pending.pop(0)
                    emit_ctx(*s2, e2)

                vst = 0
                for j, slot in enumerate(slots):
                    expt = emit_scores_exp(*slot)
                    pending.append((slot, expt))
                    if vst < NST:
                        emit_vproj(vst)
                        vst += 1
                    trail_eff = TRAIL if j < len(slots) - 34 else 2
                    for _ in range(3):
                        if not pending:
                            break
                        need = (trail_eff + GAP if pending[0][0][2] == 0
                                else trail_eff)
                        if len(pending) > need:
                            pop_one()
                        else:
                            break
                    if bg_work and j >= bg_work[0][0]:
                        bg_work.pop(0)[1]()
                    elif outproj_work and j % 2 == 0:
                        # every other slot: the PSUM->DRAM store's read
                        # completion is what frees the sm bank (~2-3us)
                        emit_outproj_tile(*outproj_work.pop(0))
                while pending:
                    pop_one()
                    if outproj_work:
                        emit_outproj_tile(*outproj_work.pop(0))
                for _, op in bg_work:
                    op()
                while outproj_work:
                    emit_outproj_tile(*outproj_work.pop(0))

    nc.compile()
    return nc


def _get_nc():
    with _lock:
        if "nc" not in _compiled:
            _compiled["nc"] = _build()
        return _compiled["nc"]


def _chan_perm():
    # fp8 DoubleRow layout: channel (p, i) <- head p//32, d = i*32 + p%32
    cols = np.empty((2, 96), np.int64)
    for i in range(2):
        for p in range(96):
            cols[i, p] = (p // 32) * 64 + i * 32 + (p % 32)
    return cols.reshape(-1)  # j = i*96 + p


def _prep_in_maps(query, key, value, prompt, Wq, bq, Wk, bk, Wv, bv, Wo, bo):
    f32 = np.float32
    qT = [np.ascontiguousarray(query[b].T).astype(BF16) for b in range(B)]
    kT = [np.ascontiguousarray(key[b].T).astype(BF16) for b in range(B)]
    vT = [np.ascontiguousarray(value[b].T).astype(BF16) for b in range(B)]
    perm = _chan_perm()
    in_maps = []
    for core in range(NCORES):
        b, g = core // NG, core % NG
        cs = slice(g * CL, (g + 1) * CL)
        Wq_g = np.asarray(Wq)[cs, :]
        Wk_g = np.asarray(Wk)[cs, :]
        bq_g = np.asarray(bq)[cs].astype(f32)
        bk_g = np.asarray(bk)[cs].astype(f32)
        kp = np.zeros((96, 2, PP), FP8)
        for i in range(2):
            for p in range(96):
                gh = g * HL + p // 32
                d = i * 32 + p % 32
                kp[p, i, :] = prompt[b, 0, :, gh, d].astype(FP8)
        vpa = np.zeros((PP, HL, D + 1), BF16)
        vpa[:, :, D] = 1.0
        for h in range(HL):
            gh = g * HL + h
            vpa[:, h, 0:D] = prompt[b, 1, :, gh, :].astype(BF16)
        in_maps.append({
            "xqT": qT[b], "xkT": kT[b], "xvT": vT[b],
            "wqT": np.ascontiguousarray(Wq_g[perm, :].T).astype(BF16),
            "wkT": np.ascontiguousarray(Wk_g[perm, :].T).astype(BF16),
            "wvT": np.ascontiguousarray(np.asarray(Wv)[cs, :].T).astype(BF16),
            "woT": np.ascontiguousarray(np.asarray(Wo)[:, cs].T).astype(BF16),
            "bq": np.ascontiguousarray(
                bq_g[perm].reshape(2, 96).T).astype(f32),
            "bk": np.ascontiguousarray(
                bk_g[perm].reshape(2, 96).T).astype(f32),
            "bv": np.ascontiguousarray(
                np.asarray(bv)[cs]).astype(f32).reshape(1, CL),
            "kp8": kp, "vp": vpa,
        })
    return in_maps


def _combine(results, bo):
    out = np.empty((B, S, E), np.float32)
    for b in range(B):
        acc = results[b * NG]["outT"].astype(np.float32)
        for g in range(1, NG):
            acc = acc + results[b * NG + g]["outT"]
        out[b] = acc.T
    if bo is not None and np.any(bo):
        out += np.asarray(bo, np.float32)
    return out


def run(inputs, trace=False):
    """Returns (output, exec_time_ns or None)."""
    from concourse import bass_utils

    nc = _get_nc()
    in_maps = _prep_in_maps(**{k: np.asarray(v) for k, v in inputs.items()})
    bo = np.asarray(inputs["bo"])
    res = bass_utils.run_bass_kernel_spmd(
        nc, in_maps, core_ids=list(range(NCORES)), trace=trace,
    )
    return _combine(res.results, bo), res.exec_time_ns


def kernel(**inputs):
    out, _ = run(inputs)
    return out


# revision 29
# speedup vs baseline: 1.2447x; 1.0403x over previous
"""Multi-head attention (B=2,S=4096,E=768,H=12,D=64 + 16-token K/V prompt
prefix) on 8 Trainium2 NeuronCores.

Sharding: 2 batches x 4 head-groups (3 heads each). Each core computes QKV
projections for its 3 heads, full attention over its batch, and a partial
output projection (its 192 ctx channels); the host sums the 4 partials per
batch.

v2 design (vs the 485us baseline, which was jointly PE- and ScalarE-bound):
  * scores matmuls run in fp8e4m3 with MatmulPerfMode.DoubleRow (d=64 split
    as [32 partitions x 2 interleave]); 0.5 cycles/row halves scores PE time.
    q/k live only in fp8; measured end-to-end rel-err impact ~1.2e-2.
  * ctx matmul is flipped: expt [k,128q] tiles are the *stationary* operand
    and v [k,65] the moving one, so each instruction streams 65 rows instead
    of 512 -- ctx PE time halves.  The ones-column in v still accumulates
    the softmax denominator (psc column 64).
  * exp is the 1/8-scaled softmax numerator; it is load-balanced across
    ScalarE (activation Exp, scale=1/8) AND Vector/Pool engines
    (tensor_tensor pow: expt = (e^{1/8})^s with a memset base tile).
  * ctx comes out of PSUM in [q, d] orientation; normalization is a single
    per-partition tensor_scalar divide; re-transposition to [d, q] for the
    out-projection rides the idle DMA engines via XBAR dma_start_transpose
    (two heads batched per transfer to satisfy the 128-col constraint).
  * PSUM: 2x[128,1024] scores + 1x[128,2,4,128pad] ctx + 2x[128,512]
    time-multiplexed (v-proj/bg q-proj/out-proj) = 8 banks exactly.
"""

import sys
import threading

import numpy as np

if "/opt/trn_rl_repo" not in sys.path:
    sys.path.insert(0, "/opt/trn_rl_repo")

import ml_dtypes

BF16 = ml_dtypes.bfloat16
FP8 = ml_dtypes.float8_e4m3

B, S, E, H, D, PP = 2, 4096, 768, 12, 64, 16
NCORES = 8
NG = 4          # head-groups (tensor parallel)
HL = H // NG    # 3 local heads
CL = HL * D     # 192 local channels
SKV = PP + S    # 4112
NKT = S // 128  # 32 full k-tiles (prefix handled separately)
QT = 1024       # q tile width for scores/exp/ctx
NSQ = S // QT   # 4
TRAIL = 12      # ctx matmuls trail scores by this many slots
NST = S // 128  # 32 v stiles
GAP = 6
# Schraudolph exp for the DVE share: bf16 bits of exp(s/8) ~=
# int16(s*SCHR_A + SCHR_B); one fused tensor_scalar (mult,add) writing
# through an int16 bitcast of the bf16 expt tile.  ~1.8% rms relative
# error on those tiles; the Act share stays exact, so total error scales
# with sqrt(phi).  C=7.5 centers the sawtooth; +0.5 makes trunc rounding.
SCHR_A = 128 * 1.4426950408889634 / 8   # 128*log2(e)/8
SCHR_B = 16256.5 - 7.5
# exp engine assignment pattern per slot: A=ScalarE (exact), D=Vector
# (Schraudolph).  GPSIMD cannot access PSUM; DVE has no transcendentals.
EXP_PAT = "ADADADADA"

_lock = threading.Lock()
_compiled = {}


def _build():
    import concourse.bass as bass  # noqa: F401
    import concourse.mybir as mybir
    import concourse.tile as tile
    from concourse import bacc

    f32 = mybir.dt.float32
    bf16 = mybir.dt.bfloat16
    fp8 = mybir.dt.float8e4
    i16 = mybir.dt.int16
    EXP = mybir.ActivationFunctionType.Exp
    IDN = mybir.ActivationFunctionType.Identity
    DIV = mybir.AluOpType.divide
    MUL = mybir.AluOpType.mult
    ADD = mybir.AluOpType.add
    DR = mybir.MatmulPerfMode.DoubleRow

    nc = bacc.Bacc("TRN2", target_bir_lowering=False, debug=False)

    xqT = nc.dram_tensor("xqT", [E, S], bf16, kind="ExternalInput").ap()
    xkT = nc.dram_tensor("xkT", [E, S], bf16, kind="ExternalInput").ap()
    xvT = nc.dram_tensor("xvT", [E, S], bf16, kind="ExternalInput").ap()
    wqT = nc.dram_tensor("wqT", [E, CL], bf16, kind="ExternalInput").ap()
    wkT = nc.dram_tensor("wkT", [E, CL], bf16, kind="ExternalInput").ap()
    wvT = nc.dram_tensor("wvT", [E, CL], bf16, kind="ExternalInput").ap()
    woT = nc.dram_tensor("woT", [CL, E], bf16, kind="ExternalInput").ap()
    bq = nc.dram_tensor("bq", [96, 2], f32, kind="ExternalInput").ap()
    bk = nc.dram_tensor("bk", [96, 2], f32, kind="ExternalInput").ap()
    bv = nc.dram_tensor("bv", [1, CL], f32, kind="ExternalInput").ap()
    kp8 = nc.dram_tensor("kp8", [96, 2, PP], fp8, kind="ExternalInput").ap()
    vp = nc.dram_tensor("vp", [PP, HL, D + 1], bf16, kind="ExternalInput").ap()
    outT = nc.dram_tensor("outT", [E, S], f32, kind="ExternalOutput").ap()

    with tile.TileContext(nc) as tc:
        with tc.tile_pool(name="persist", bufs=1) as pers:
            # q-projection weights/bias first: they gate the first matmuls
            wq_sb = pers.tile([128, 6, CL], bf16)
            nc.sync.dma_start(wq_sb[:], wqT.rearrange("(t p) c -> p t c", p=128))
            bq_sb = pers.tile([96, 2], f32)
            nc.sync.dma_start(bq_sb[:], bq[:])

            wk_sb = pers.tile([128, 6, CL], bf16)
            wv_sb = pers.tile([128, 6, CL], bf16)
            wo_sb = pers.tile([128, 2, E], bf16)
            bk_sb = pers.tile([96, 2], f32)
            bvb_sb = pers.tile([128, CL], f32)
            kp_sb = pers.tile([96, 2, PP], fp8)
            vp_sb = pers.tile([PP, HL, D + 1], bf16)

            # activations
            qT8 = pers.tile([96, 2, S], fp8)
            kT8 = pers.tile([96, 2, S], fp8)
            v_sb = pers.tile([128, NST, HL, D + 1], bf16)
            ctxT_sb = pers.tile([128, 2, S], bf16)
            expp_sb = pers.tile([PP, HL, S], bf16)  # prefix exp rows per head
            # normalized ctx staging, [q, d] orientation, manual sq-parity
            # double buffer; cn01 packs heads 0,1 so one XBAR dma transposes
            # 128 columns at once; cn2 pads head 2 with a junk half.
            cn01 = pers.tile([128, 2, 8, 2, D], bf16)
            cn2 = pers.tile([128, 2, 8, 2, D], bf16)

            nc.vector.memset(v_sb[:, :, :, D:D + 1], 1.0)
            nc.vector.memset(cn2[:], 0.0)

            # One unified stream phase.  PSUM: ps_s 3x[128,1024] (6 banks,
            # 3-deep rotation so the exp engines run back-to-back) + ps_c
            # 1x[128,2,4,128] (2 banks) = 8.  Projections / out-projection /
            # prefix borrow ps_s rotation slots (same tag+shape, partial use);
            # note matmul PSUM writes must stay within one 2KB bank, so every
            # matmul writes at most 512 f32 columns.
            with (
                tc.tile_pool(name="ps_s", bufs=3, space="PSUM") as ps_s,
                tc.tile_pool(name="ps_c", bufs=1, space="PSUM") as ps_c,
                tc.tile_pool(name="expt_pool", bufs=20) as expt_pool,
                tc.tile_pool(name="xv_pool", bufs=8) as xv_pool,
                tc.tile_pool(name="xq2_pool", bufs=13) as xq2_pool,
                tc.tile_pool(name="out_pool", bufs=3) as out_pool,
                tc.tile_pool(name="nrm_pool", bufs=2) as nrm_pool,
            ):
                expcnt = [0]

                def emit_exp(dst, src, exact=False):
                    eng = "A" if exact else EXP_PAT[expcnt[0] % len(EXP_PAT)]
                    expcnt[0] += 1
                    if eng == "A":
                        nc.scalar.activation(dst, src, EXP, scale=0.125)
                    else:
                        nc.vector.tensor_scalar(
                            dst.bitcast(i16), src, float(SCHR_A),
                            float(SCHR_B), MUL, ADD)

                def pss_tile(name):
                    return ps_s.tile([128, QT], f32, tag="pss", name=name)

                def load_x_chunks(xin, sq):
                    tiles = []
                    for ech in range(6):
                        xt = xq2_pool.tile([128, QT], bf16, tag="xt2",
                                           name="xt2")
                        nc.sync.dma_start(
                            xt[:],
                            xin[ech * 128:(ech + 1) * 128,
                                sq * QT:(sq + 1) * QT],
                        )
                        tiles.append(xt)
                    return tiles

                def emit_proj_group(xts, wsb, bsb, dst8, sq, i):
                    # one [96, 1024] projection result via two 512-col mms
                    p = pss_tile("pqk")
                    for n in range(2):
                        ns = slice(n * 512, (n + 1) * 512)
                        for ech in range(6):
                            nc.tensor.matmul(
                                p[0:96, ns],
                                wsb[:, ech, i * 96:(i + 1) * 96],
                                xts[ech][:, ns],
                                start=(ech == 0), stop=(ech == 5),
                            )
                    qs = slice(sq * QT, (sq + 1) * QT)
                    nc.scalar.activation(
                        dst8[0:96, i, qs], p[0:96, :], IDN,
                        bias=bsb[:, i:i + 1])

                def emit_prefix(sq, h):
                    hp = slice(32 * h, 32 * h + 32)
                    psp = pss_tile("psp")
                    for n in range(2):
                        ns = slice(n * 512, (n + 1) * 512)
                        qs = slice(sq * QT + n * 512, sq * QT + (n + 1) * 512)
                        nc.tensor.matmul(
                            psp[0:PP, ns], kp_sb[hp, :, :], qT8[hp, :, qs],
                            start=True, stop=True, perf_mode=DR,
                        )
                    emit_exp(expp_sb[:, h, sq * QT:(sq + 1) * QT],
                             psp[0:PP, :], exact=True)

                # xv DMA loads, one sq-group of 6 chunks at a time
                xvts = {}

                def load_xv(sqx):
                    tiles = []
                    for ech in range(6):
                        xvt = xv_pool.tile([128, QT], bf16, tag="xvt",
                                           name="xvt")
                        nc.sync.dma_start(
                            xvt[:],
                            xvT[ech * 128:(ech + 1) * 128,
                                sqx * QT:(sqx + 1) * QT],
                        )
                        tiles.append(xvt)
                    xvts[sqx] = tiles

                def emit_vproj(st):
                    sqx, stl = st // (QT // 128), st % (QT // 128)
                    if st == 0:
                        load_xv(0)
                    if stl == 0 and sqx + 1 < NSQ:
                        load_xv(sqx + 1)
                    pv = pss_tile("pv")
                    for ech in range(6):
                        nc.tensor.matmul(
                            pv[:, 0:CL],
                            xvts[sqx][ech][:, stl * 128:(stl + 1) * 128],
                            wv_sb[:, ech, :],
                            start=(ech == 0), stop=(ech == 5),
                        )
                    nc.vector.tensor_add(
                        v_sb[:, st, :, 0:D],
                        pv[:, 0:CL].rearrange("p (h d) -> p h d", h=HL),
                        bvb_sb[:].rearrange("p (h d) -> p h d", h=HL),
                    )
                    if stl == (QT // 128) - 1:
                        del xvts[sqx]

                def emit_scores_exp(sq, h, kt):
                    hp = slice(32 * h, 32 * h + 32)
                    expt = expt_pool.tile([128, QT], bf16, tag="expt",
                                          name="expt")
                    pss = pss_tile("pss")
                    for n in range(2):
                        ns = slice(n * 512, (n + 1) * 512)
                        qs = slice(sq * QT + n * 512, sq * QT + (n + 1) * 512)
                        nc.tensor.matmul(
                            pss[:, ns],
                            kT8[hp, :, kt * 128:(kt + 1) * 128],
                            qT8[hp, :, qs],
                            start=True, stop=True, perf_mode=DR,
                        )
                    emit_exp(expt[:], pss[:])
                    return expt

                psc_tiles = {}
                outproj_work = []

                def emit_ctx(sq, h, kt, expt):
                    key = (sq, h)
                    if kt == 0:
                        psc_tiles[key] = ps_c.tile(
                            [128, 2, 4, 128], f32, tag="psc", name="psc")
                    psc = psc_tiles[key]
                    # PSUM zero-region (2KB bank) semantics: only the first
                    # slice per bank may carry start=True (it marks the whole
                    # region pending-zero; sibling slices' first writes then
                    # overwrite-on-first-touch), and only the last slice may
                    # carry stop=True (it clears the whole region's group).
                    for qb in range(8):
                        nc.tensor.matmul(
                            psc[:, qb // 4, qb % 4, 0:D + 1],
                            expt[:, qb * 128:(qb + 1) * 128],
                            v_sb[:, kt, h, :],
                            start=(kt == 0 and qb % 4 == 0),
                            stop=(kt == NKT - 1 and qb % 4 == 3),
                        )
                    if kt == TRAIL - 1:
                        # prompt-prefix ctx contribution (reads expp_sb rows)
                        for qb in range(8):
                            qs = slice(sq * QT + qb * 128,
                                       sq * QT + (qb + 1) * 128)
                            nc.tensor.matmul(
                                psc[:, qb // 4, qb % 4, 0:D + 1],
                                expp_sb[:, h, qs],
                                vp_sb[:, h, :],
                                start=False, stop=False,
                            )
                    if kt == NKT - 1:
                        emit_norm(sq, h, psc)
                        del psc_tiles[key]

                def emit_norm(sq, h, psc):
                    par = sq % 2
                    cn = cn2 if h == 2 else cn01
                    hh = 0 if h == 2 else h
                    # hw tensor_scalar has no divide: batched reciprocal of
                    # the 8 denominator columns, then per-block multiplies
                    rc = nrm_pool.tile([128, 8], f32, tag="rc", name="rc")
                    nc.vector.reciprocal(
                        rc[:].rearrange("p (a b) -> p a b", a=2),
                        psc[:, :, :, D:D + 1].squeeze(3))
                    for qb in range(8):
                        nc.vector.tensor_scalar(
                            cn[:, par, qb, hh, :],
                            psc[:, qb // 4, qb % 4, 0:D],
                            rc[:, qb:qb + 1],
                            None, MUL,
                        )
                    if h >= 1:
                        # heads 0,1 pair (after h1) / head 2 -> XBAR transpose
                        cnin, pr = (cn01, 0) if h == 1 else (cn2, 1)
                        for qb in range(8):
                            qs = slice(sq * QT + qb * 128,
                                       sq * QT + (qb + 1) * 128)
                            nc.sync.dma_start_transpose(
                                ctxT_sb[:, pr, qs], cnin[:, par, qb, :, :])
                    if h == HL - 1:
                        for et in range(6):
                            outproj_work.append((et, sq))

                def emit_outproj_tile(et, sq):
                    es = slice(et * 128, (et + 1) * 128)
                    po3 = pss_tile("po3")
                    for n in range(2):
                        ns = slice(n * 512, (n + 1) * 512)
                        qs = slice(sq * QT + n * 512, sq * QT + (n + 1) * 512)
                        nc.tensor.matmul(
                            po3[:, ns], wo_sb[:, 0, es], ctxT_sb[:, 0, qs],
                            start=True, stop=False,
                        )
                        nc.tensor.matmul(
                            po3[:, ns], wo_sb[0:64, 1, es],
                            ctxT_sb[0:64, 1, qs],
                            start=False, stop=True,
                        )
                    # DMA cannot read PSUM: stage through SBUF via DVE
                    ot = out_pool.tile([128, QT], f32, tag="ot", name="ot")
                    nc.vector.tensor_copy(ot[:], po3[:])
                    nc.sync.dma_start(
                        outT[es, sq * QT:(sq + 1) * QT], ot[:])

                # ---- stream startup: q-proj(sq0), prefix(sq0), k-proj(0) ----
                xq0 = load_x_chunks(xqT, 0)
                nc.sync.dma_start(
                    wk_sb[:], wkT.rearrange("(t p) c -> p t c", p=128))
                nc.sync.dma_start(bk_sb[:], bk[:])
                nc.sync.dma_start(kp_sb[:], kp8[:])
                for i in range(2):
                    emit_proj_group(xq0, wq_sb, bq_sb, qT8, 0, i)
                xk = {0: load_x_chunks(xkT, 0)}
                nc.sync.dma_start(
                    wv_sb[:], wvT.rearrange("(t p) c -> p t c", p=128))
                nc.sync.dma_start(bvb_sb[:], bv.to_broadcast((128, CL)))
                nc.sync.dma_start(vp_sb[:], vp[:])
                nc.sync.dma_start(wo_sb[:, 0, :], woT[0:128, :])
                nc.sync.dma_start(wo_sb[0:64, 1, :], woT[128:CL, :])
                for h in range(HL):
                    emit_prefix(0, h)
                for i in range(2):
                    emit_proj_group(xk[0], wk_sb, bk_sb, kT8, 0, i)

                # deferred ops drained into designated slots:
                # k-proj blocks 1..3 feed the first head's kt sweep just in
                # time; q-proj for sq 1..3 + their prefixes run mid-stream.
                bg_work = []
                for b in (1, 2, 3):
                    bg_work.append((8 * b - 6, lambda b=b: xk.__setitem__(
                        b, load_x_chunks(xkT, b))))
                    for i in range(2):
                        bg_work.append(
                            (8 * b - 4 + 2 * i,
                             lambda b=b, i=i: emit_proj_group(
                                 xk[b], wk_sb, bk_sb, kT8, b, i)))
                xq = {}
                for nb, sqb in ((40, 1), (136, 2), (232, 3)):
                    bg_work.append((nb, lambda s=sqb: xq.__setitem__(
                        s, load_x_chunks(xqT, s))))
                    for i in range(2):
                        bg_work.append(
                            (nb + 5 + 5 * i,
                             lambda s=sqb, i=i: emit_proj_group(
                                 xq[s], wq_sb, bq_sb, qT8, s, i)))
                    for h in range(HL):
                        bg_work.append(
                            (nb + 15 + 5 * h,
                             lambda s=sqb, h=h: emit_prefix(s, h)))

                slots = [(sq, h, kt)
                         for sq in range(NSQ)
                         for h in range(HL)
                         for kt in range(NKT)]
                pending = []

                def pop_one():
                    (s2, e2) = pending.pop(0)
                    emit_ctx(*s2, e2)

                vst = 0
                for j, slot in enumerate(slots):
                    expt = emit_scores_exp(*slot)
                    pending.append((slot, expt))
                    if vst < NST:
                        emit_vproj(vst)
                        vst += 1
                    trail_eff = TRAIL if j < len(slots) - 34 else 2
                    for _ in range(3):
                        if not pending:
                            break
                        need = (trail_eff + GAP if pending[0][0][2] == 0
                                else trail_eff)
                        if len(pending) > need:
                            pop_one()
                        else:
                            break
                    if bg_work and j >= bg_work[0][0]:
                        bg_work.pop(0)[1]()
                    elif outproj_work and j % 2 == 0:
                        # every other slot: out-proj rides the pss rotation
                        emit_outproj_tile(*outproj_work.pop(0))
                while pending:
                    pop_one()
                    if outproj_work:
                        emit_outproj_tile(*outproj_work.pop(0))
                for _, op in bg_work:
                    op()
                while outproj_work:
                    emit_outproj_tile(*outproj_work.pop(0))

    nc.compile()
    return nc


def _get_nc():
    with _lock:
        if "nc" not in _compiled:
            _compiled["nc"] = _build()
        return _compiled["nc"]


def _chan_perm():
    # fp8 DoubleRow layout: channel (p, i) <- head p//32, d = i*32 + p%32
    cols = np.empty((2, 96), np.int64)
    for i in range(2):
        for p in range(96):
            cols[i, p] = (p // 32) * 64 + i * 32 + (p % 32)
    return cols.reshape(-1)  # j = i*96 + p


def _prep_in_maps(query, key, value, prompt, Wq, bq, Wk, bk, Wv, bv, Wo, bo):
    f32 = np.float32
    qT = [np.ascontiguousarray(query[b].T).astype(BF16) for b in range(B)]
    kT = [np.ascontiguousarray(key[b].T).astype(BF16) for b in range(B)]
    vT = [np.ascontiguousarray(value[b].T).astype(BF16) for b in range(B)]
    perm = _chan_perm()
    in_maps = []
    for core in range(NCORES):
        b, g = core // NG, core % NG
        cs = slice(g * CL, (g + 1) * CL)
        Wq_g = np.asarray(Wq)[cs, :]
        Wk_g = np.asarray(Wk)[cs, :]
        bq_g = np.asarray(bq)[cs].astype(f32)
        bk_g = np.asarray(bk)[cs].astype(f32)
        kp = np.zeros((96, 2, PP), FP8)
        for i in range(2):
            for p in range(96):
                gh = g * HL + p // 32
                d = i * 32 + p % 32
                kp[p, i, :] = prompt[b, 0, :, gh, d].astype(FP8)
        vpa = np.zeros((PP, HL, D + 1), BF16)
        vpa[:, :, D] = 1.0
        for h in range(HL):
            gh = g * HL + h
            vpa[:, h, 0:D] = prompt[b, 1, :, gh, :].astype(BF16)
        in_maps.append({
            "xqT": qT[b], "xkT": kT[b], "xvT": vT[b],
            "wqT": np.ascontiguousarray(Wq_g[perm, :].T).astype(BF16),
            "wkT": np.ascontiguousarray(Wk_g[perm, :].T).astype(BF16),
            "wvT": np.ascontiguousarray(np.asarray(Wv)[cs, :].T).astype(BF16),
            "woT": np.ascontiguousarray(np.asarray(Wo)[:, cs].T).astype(BF16),
            "bq": np.ascontiguousarray(
                bq_g[perm].reshape(2, 96).T).astype(f32),
            "bk": np.ascontiguousarray(
                bk_g[perm].reshape(2, 96).T).astype(f32),
            "bv": np.ascontiguousarray(
                np.asarray(bv)[cs]).astype(f32).reshape(1, CL),
            "kp8": kp, "vp": vpa,
        })
    return in_maps


def _combine(results, bo):
    out = np.empty((B, S, E), np.float32)
    for b in range(B):
        acc = results[b * NG]["outT"].astype(np.float32)
        for g in range(1, NG):
            acc = acc + results[b * NG + g]["outT"]
        out[b] = acc.T
    if bo is not None and np.any(bo):
        out += np.asarray(bo, np.float32)
    return out


def run(inputs, trace=False):
    """Returns (output, exec_time_ns or None)."""
    from concourse import bass_utils

    nc = _get_nc()
    in_maps = _prep_in_maps(**{k: np.asarray(v) for k, v in inputs.items()})
    bo = np.asarray(inputs["bo"])
    res = bass_utils.run_bass_kernel_spmd(
        nc, in_maps, core_ids=list(range(NCORES)), trace=trace,
    )
    return _combine(res.results, bo), res.exec_time_ns


def kernel(**inputs):
    out, _ = run(inputs)
    return out


# revision 33
# speedup vs baseline: 1.2642x; 1.0156x over previous
"""Multi-head attention (B=2,S=4096,E=768,H=12,D=64 + 16-token K/V prompt
prefix) on 8 Trainium2 NeuronCores.

Sharding: 2 batches x 4 head-groups (3 heads each). Each core computes QKV
projections for its 3 heads, full attention over its batch, and a partial
output projection (its 192 ctx channels); the host sums the 4 partials per
batch.

v2 design (vs the 485us baseline, which was jointly PE- and ScalarE-bound):
  * scores matmuls run in fp8e4m3 with MatmulPerfMode.DoubleRow (d=64 split
    as [32 partitions x 2 interleave]); 0.5 cycles/row halves scores PE time.
    q/k live only in fp8; measured end-to-end rel-err impact ~1.2e-2.
  * ctx matmul is flipped: expt [k,128q] tiles are the *stationary* operand
    and v [k,65] the moving one, so each instruction streams 65 rows instead
    of 512 -- ctx PE time halves.  The ones-column in v still accumulates
    the softmax denominator (psc column 64).
  * exp is the 1/8-scaled softmax numerator; it is load-balanced across
    ScalarE (activation Exp, scale=1/8) AND Vector/Pool engines
    (tensor_tensor pow: expt = (e^{1/8})^s with a memset base tile).
  * ctx comes out of PSUM in [q, d] orientation; normalization is a single
    per-partition tensor_scalar divide; re-transposition to [d, q] for the
    out-projection rides the idle DMA engines via XBAR dma_start_transpose
    (two heads batched per transfer to satisfy the 128-col constraint).
  * PSUM: 2x[128,1024] scores + 1x[128,2,4,128pad] ctx + 2x[128,512]
    time-multiplexed (v-proj/bg q-proj/out-proj) = 8 banks exactly.
"""

import sys
import threading

import numpy as np

if "/opt/trn_rl_repo" not in sys.path:
    sys.path.insert(0, "/opt/trn_rl_repo")

import ml_dtypes

BF16 = ml_dtypes.bfloat16
FP8 = ml_dtypes.float8_e4m3

B, S, E, H, D, PP = 2, 4096, 768, 12, 64, 16
NCORES = 8
NG = 4          # head-groups (tensor parallel)
HL = H // NG    # 3 local heads
CL = HL * D     # 192 local channels
SKV = PP + S    # 4112
NKT = S // 128  # 32 full k-tiles (prefix handled separately)
QT = 1024       # q tile width for scores/exp/ctx
NSQ = S // QT   # 4
TRAIL = 12      # ctx matmuls trail scores by this many slots
NST = S // 128  # 32 v stiles
GAP = 6
# Schraudolph exp for the DVE share: bf16 bits of exp(s/8) ~=
# int16(s*SCHR_A + SCHR_B); one fused tensor_scalar (mult,add) writing
# through an int16 bitcast of the bf16 expt tile.  ~1.8% rms relative
# error on those tiles; the Act share stays exact, so total error scales
# with sqrt(phi).  C=7.5 centers the sawtooth; +0.5 makes trunc rounding.
SCHR_A = 128 * 1.4426950408889634 / 8   # 128*log2(e)/8
SCHR_B = 16256.5 - 7.5
# exp engine assignment pattern per slot: A=ScalarE (exact), D=Vector
# (Schraudolph).  GPSIMD cannot access PSUM; DVE has no transcendentals.
EXP_PAT = "ADADADADA"

_lock = threading.Lock()
_compiled = {}


def _build():
    import concourse.bass as bass  # noqa: F401
    import concourse.mybir as mybir
    import concourse.tile as tile
    from concourse import bacc

    f32 = mybir.dt.float32
    bf16 = mybir.dt.bfloat16
    fp8 = mybir.dt.float8e4
    i16 = mybir.dt.int16
    EXP = mybir.ActivationFunctionType.Exp
    IDN = mybir.ActivationFunctionType.Identity
    DIV = mybir.AluOpType.divide
    MUL = mybir.AluOpType.mult
    ADD = mybir.AluOpType.add
    DR = mybir.MatmulPerfMode.DoubleRow

    nc = bacc.Bacc("TRN2", target_bir_lowering=False, debug=False)

    xqT = nc.dram_tensor("xqT", [E, S], bf16, kind="ExternalInput").ap()
    xkT = nc.dram_tensor("xkT", [E, S], bf16, kind="ExternalInput").ap()
    xvT = nc.dram_tensor("xvT", [E, S], bf16, kind="ExternalInput").ap()
    wqT = nc.dram_tensor("wqT", [E, CL], bf16, kind="ExternalInput").ap()
    wkT = nc.dram_tensor("wkT", [E, CL], bf16, kind="ExternalInput").ap()
    wvT = nc.dram_tensor("wvT", [E, CL], bf16, kind="ExternalInput").ap()
    woT = nc.dram_tensor("woT", [CL, E], bf16, kind="ExternalInput").ap()
    bq = nc.dram_tensor("bq", [96, 2], f32, kind="ExternalInput").ap()
    bk = nc.dram_tensor("bk", [96, 2], f32, kind="ExternalInput").ap()
    bv = nc.dram_tensor("bv", [1, CL], f32, kind="ExternalInput").ap()
    kp8 = nc.dram_tensor("kp8", [96, 2, PP], fp8, kind="ExternalInput").ap()
    vp = nc.dram_tensor("vp", [PP, HL, D + 1], bf16, kind="ExternalInput").ap()
    outT = nc.dram_tensor("outT", [E, S], f32, kind="ExternalOutput").ap()

    with tile.TileContext(nc) as tc:
        with tc.tile_pool(name="persist", bufs=1) as pers:
            # q-projection weights/bias first: they gate the first matmuls
            wq_sb = pers.tile([128, 6, CL], bf16)
            nc.sync.dma_start(wq_sb[:], wqT.rearrange("(t p) c -> p t c", p=128))
            bq_sb = pers.tile([96, 2], f32)
            nc.sync.dma_start(bq_sb[:], bq[:])

            wk_sb = pers.tile([128, 6, CL], bf16)
            wv_sb = pers.tile([128, 6, CL], bf16)
            wo_sb = pers.tile([128, 2, E], bf16)
            bk_sb = pers.tile([96, 2], f32)
            bvb_sb = pers.tile([128, CL], f32)
            kp_sb = pers.tile([96, 2, PP], fp8)
            vp_sb = pers.tile([PP, HL, D + 1], bf16)

            # activations
            qT8 = pers.tile([96, 2, S], fp8)
            kT8 = pers.tile([96, 2, S], fp8)
            v_sb = pers.tile([128, NST, HL, D + 1], bf16)
            ctxT_sb = pers.tile([128, 2, S], bf16)
            expp_sb = pers.tile([PP, HL, S], bf16)  # prefix exp rows per head
            # normalized ctx staging, [q, d] orientation, manual sq-parity
            # double buffer; cn01 packs heads 0,1 so one XBAR dma transposes
            # 128 columns at once; cn2 pads head 2 with a junk half.
            cn01 = pers.tile([128, 2, 8, 2, D], bf16)
            cn2 = pers.tile([128, 2, 8, 2, D], bf16)

            nc.vector.memset(v_sb[:, :, :, D:D + 1], 1.0)
            nc.vector.memset(cn2[:], 0.0)

            # One unified stream phase.  PSUM: ps_s 3x[128,1024] (6 banks,
            # 3-deep rotation so the exp engines run back-to-back) + ps_c
            # 1x[128,2,4,128] (2 banks) = 8.  Projections / out-projection /
            # prefix borrow ps_s rotation slots (same tag+shape, partial use);
            # note matmul PSUM writes must stay within one 2KB bank, so every
            # matmul writes at most 512 f32 columns.
            with (
                tc.tile_pool(name="ps_s", bufs=3, space="PSUM") as ps_s,
                tc.tile_pool(name="ps_c", bufs=1, space="PSUM") as ps_c,
                tc.tile_pool(name="expt_pool", bufs=20) as expt_pool,
                tc.tile_pool(name="xv_pool", bufs=8) as xv_pool,
                tc.tile_pool(name="xq2_pool", bufs=13) as xq2_pool,
                tc.tile_pool(name="out_pool", bufs=3) as out_pool,
                tc.tile_pool(name="nrm_pool", bufs=2) as nrm_pool,
            ):
                expcnt = [0]

                def emit_exp(dst, src, exact=False):
                    eng = "A" if exact else EXP_PAT[expcnt[0] % len(EXP_PAT)]
                    expcnt[0] += 1
                    if eng == "A":
                        nc.scalar.activation(dst, src, EXP, scale=0.125)
                    else:
                        nc.vector.tensor_scalar(
                            dst.bitcast(i16), src, float(SCHR_A),
                            float(SCHR_B), MUL, ADD)

                def pss_tile(name):
                    return ps_s.tile([128, QT], f32, tag="pss", name=name)

                def load_x_chunks(xin, sq):
                    tiles = []
                    for ech in range(6):
                        xt = xq2_pool.tile([128, QT], bf16, tag="xt2",
                                           name="xt2")
                        nc.sync.dma_start(
                            xt[:],
                            xin[ech * 128:(ech + 1) * 128,
                                sq * QT:(sq + 1) * QT],
                        )
                        tiles.append(xt)
                    return tiles

                def emit_proj_group(xts, wsb, bsb, dst8, sq, i):
                    # one [96, 1024] projection result via two 512-col mms
                    p = pss_tile("pqk")
                    for n in range(2):
                        ns = slice(n * 512, (n + 1) * 512)
                        for ech in range(6):
                            nc.tensor.matmul(
                                p[0:96, ns],
                                wsb[:, ech, i * 96:(i + 1) * 96],
                                xts[ech][:, ns],
                                start=(ech == 0), stop=(ech == 5),
                            )
                    qs = slice(sq * QT, (sq + 1) * QT)
                    nc.scalar.activation(
                        dst8[0:96, i, qs], p[0:96, :], IDN,
                        bias=bsb[:, i:i + 1])

                def emit_prefix(sq, h):
                    hp = slice(32 * h, 32 * h + 32)
                    psp = pss_tile("psp")
                    for n in range(2):
                        ns = slice(n * 512, (n + 1) * 512)
                        qs = slice(sq * QT + n * 512, sq * QT + (n + 1) * 512)
                        nc.tensor.matmul(
                            psp[0:PP, ns], kp_sb[hp, :, :], qT8[hp, :, qs],
                            start=True, stop=True, perf_mode=DR,
                        )
                    emit_exp(expp_sb[:, h, sq * QT:(sq + 1) * QT],
                             psp[0:PP, :], exact=True)

                # xv DMA loads, one sq-group of 6 chunks at a time
                xvts = {}

                def load_xv(sqx):
                    tiles = []
                    for ech in range(6):
                        xvt = xv_pool.tile([128, QT], bf16, tag="xvt",
                                           name="xvt")
                        nc.sync.dma_start(
                            xvt[:],
                            xvT[ech * 128:(ech + 1) * 128,
                                sqx * QT:(sqx + 1) * QT],
                        )
                        tiles.append(xvt)
                    xvts[sqx] = tiles

                def emit_vproj(st):
                    sqx, stl = st // (QT // 128), st % (QT // 128)
                    if st == 0:
                        load_xv(0)
                    # prefetch mid-group so the xv DMAs stay off the k-proj
                    # chunks' critical DMA window at stream start
                    if stl == 4 and sqx + 1 < NSQ:
                        load_xv(sqx + 1)
                    pv = pss_tile("pv")
                    for ech in range(6):
                        nc.tensor.matmul(
                            pv[:, 0:CL],
                            xvts[sqx][ech][:, stl * 128:(stl + 1) * 128],
                            wv_sb[:, ech, :],
                            start=(ech == 0), stop=(ech == 5),
                        )
                    nc.vector.tensor_add(
                        v_sb[:, st, :, 0:D],
                        pv[:, 0:CL].rearrange("p (h d) -> p h d", h=HL),
                        bvb_sb[:].rearrange("p (h d) -> p h d", h=HL),
                    )
                    if stl == (QT // 128) - 1:
                        del xvts[sqx]

                def emit_scores_exp(sq, h, kt):
                    hp = slice(32 * h, 32 * h + 32)
                    expt = expt_pool.tile([128, QT], bf16, tag="expt",
                                          name="expt")
                    pss = pss_tile("pss")
                    for n in range(2):
                        ns = slice(n * 512, (n + 1) * 512)
                        qs = slice(sq * QT + n * 512, sq * QT + (n + 1) * 512)
                        nc.tensor.matmul(
                            pss[:, ns],
                            kT8[hp, :, kt * 128:(kt + 1) * 128],
                            qT8[hp, :, qs],
                            start=True, stop=True, perf_mode=DR,
                        )
                    emit_exp(expt[:], pss[:])
                    return expt

                psc_tiles = {}
                outproj_work = []

                def emit_ctx(sq, h, kt, expt):
                    key = (sq, h)
                    if kt == 0:
                        psc_tiles[key] = ps_c.tile(
                            [128, 2, 4, 128], f32, tag="psc", name="psc")
                    psc = psc_tiles[key]
                    # PSUM zero-region (2KB bank) semantics: only the first
                    # slice per bank may carry start=True (it marks the whole
                    # region pending-zero; sibling slices' first writes then
                    # overwrite-on-first-touch), and only the last slice may
                    # carry stop=True (it clears the whole region's group).
                    for qb in range(8):
                        nc.tensor.matmul(
                            psc[:, qb // 4, qb % 4, 0:D + 1],
                            expt[:, qb * 128:(qb + 1) * 128],
                            v_sb[:, kt, h, :],
                            start=(kt == 0 and qb % 4 == 0),
                            stop=(kt == NKT - 1 and qb % 4 == 3),
                        )
                    if kt == TRAIL - 1:
                        # prompt-prefix ctx contribution (reads expp_sb rows)
                        for qb in range(8):
                            qs = slice(sq * QT + qb * 128,
                                       sq * QT + (qb + 1) * 128)
                            nc.tensor.matmul(
                                psc[:, qb // 4, qb % 4, 0:D + 1],
                                expp_sb[:, h, qs],
                                vp_sb[:, h, :],
                                start=False, stop=False,
                            )
                    if kt == NKT - 1:
                        emit_norm(sq, h, psc)
                        del psc_tiles[key]

                def emit_norm(sq, h, psc):
                    par = sq % 2
                    cn = cn2 if h == 2 else cn01
                    hh = 0 if h == 2 else h
                    # hw tensor_scalar has no divide: batched reciprocal of
                    # the 8 denominator columns, then per-block multiplies,
                    # alternating Act/DVE so neither engine queue bursts
                    rc = nrm_pool.tile([128, 8], f32, tag="rc", name="rc")
                    nc.vector.reciprocal(
                        rc[:].rearrange("p (a b) -> p a b", a=2),
                        psc[:, :, :, D:D + 1].squeeze(3))
                    for qb in range(8):
                        if qb % 2 == 0:
                            nc.scalar.activation(
                                cn[:, par, qb, hh, :],
                                psc[:, qb // 4, qb % 4, 0:D],
                                IDN, scale=rc[:, qb:qb + 1])
                        else:
                            nc.vector.tensor_scalar(
                                cn[:, par, qb, hh, :],
                                psc[:, qb // 4, qb % 4, 0:D],
                                rc[:, qb:qb + 1],
                                None, MUL,
                            )
                    if h >= 1:
                        # heads 0,1 pair (after h1) / head 2 -> XBAR transpose
                        cnin, pr = (cn01, 0) if h == 1 else (cn2, 1)
                        for qb in range(8):
                            qs = slice(sq * QT + qb * 128,
                                       sq * QT + (qb + 1) * 128)
                            nc.sync.dma_start_transpose(
                                ctxT_sb[:, pr, qs], cnin[:, par, qb, :, :])
                    if h == HL - 1:
                        for et in range(6):
                            outproj_work.append((et, sq))

                def emit_outproj_tile(et, sq):
                    es = slice(et * 128, (et + 1) * 128)
                    po3 = pss_tile("po3")
                    for n in range(2):
                        ns = slice(n * 512, (n + 1) * 512)
                        qs = slice(sq * QT + n * 512, sq * QT + (n + 1) * 512)
                        nc.tensor.matmul(
                            po3[:, ns], wo_sb[:, 0, es], ctxT_sb[:, 0, qs],
                            start=True, stop=False,
                        )
                        nc.tensor.matmul(
                            po3[:, ns], wo_sb[0:64, 1, es],
                            ctxT_sb[0:64, 1, qs],
                            start=False, stop=True,
                        )
                    # DMA cannot read PSUM: stage through SBUF, alternating
                    # the evac engine so neither queue bursts at sq ends
                    ot = out_pool.tile([128, QT], f32, tag="ot", name="ot")
                    if et % 2 == 0:
                        nc.scalar.activation(ot[:], po3[:], IDN)
                    else:
                        nc.vector.tensor_copy(ot[:], po3[:])
                    nc.sync.dma_start(
                        outT[es, sq * QT:(sq + 1) * QT], ot[:])

                # ---- stream startup: q-proj(sq0), prefix(sq0), k-proj(0) ----
                xq0 = load_x_chunks(xqT, 0)
                nc.sync.dma_start(
                    wk_sb[:], wkT.rearrange("(t p) c -> p t c", p=128))
                nc.sync.dma_start(bk_sb[:], bk[:])
                nc.sync.dma_start(kp_sb[:], kp8[:])
                for i in range(2):
                    emit_proj_group(xq0, wq_sb, bq_sb, qT8, 0, i)
                xk = {0: load_x_chunks(xkT, 0)}
                nc.sync.dma_start(
                    wv_sb[:], wvT.rearrange("(t p) c -> p t c", p=128))
                nc.sync.dma_start(bvb_sb[:], bv.to_broadcast((128, CL)))
                nc.sync.dma_start(vp_sb[:], vp[:])
                nc.sync.dma_start(wo_sb[:, 0, :], woT[0:128, :])
                nc.sync.dma_start(wo_sb[0:64, 1, :], woT[128:CL, :])
                for h in range(HL):
                    emit_prefix(0, h)
                for i in range(2):
                    emit_proj_group(xk[0], wk_sb, bk_sb, kT8, 0, i)

                # deferred ops drained into designated slots:
                # k-proj blocks 1..3 feed the first head's kt sweep just in
                # time; q-proj for sq 1..3 + their prefixes run mid-stream.
                bg_work = []
                for b in (1, 2, 3):
                    bg_work.append((8 * b - 6, lambda b=b: xk.__setitem__(
                        b, load_x_chunks(xkT, b))))
                    for i in range(2):
                        bg_work.append(
                            (8 * b - 4 + 2 * i,
                             lambda b=b, i=i: emit_proj_group(
                                 xk[b], wk_sb, bk_sb, kT8, b, i)))
                xq = {}
                for nb, sqb in ((40, 1), (136, 2), (232, 3)):
                    bg_work.append((nb, lambda s=sqb: xq.__setitem__(
                        s, load_x_chunks(xqT, s))))
                    for i in range(2):
                        bg_work.append(
                            (nb + 5 + 5 * i,
                             lambda s=sqb, i=i: emit_proj_group(
                                 xq[s], wq_sb, bq_sb, qT8, s, i)))
                    for h in range(HL):
                        bg_work.append(
                            (nb + 15 + 5 * h,
                             lambda s=sqb, h=h: emit_prefix(s, h)))

                slots = [(sq, h, kt)
                         for sq in range(NSQ)
                         for h in range(HL)
                         for kt in range(NKT)]
                pending = []

                def pop_one():
                    (s2, e2) = pending.pop(0)
                    emit_ctx(*s2, e2)

                vst = 0
                for j, slot in enumerate(slots):
                    expt = emit_scores_exp(*slot)
                    pending.append((slot, expt))
                    if vst < NST:
                        emit_vproj(vst)
                        vst += 1
                    trail_eff = TRAIL if j < len(slots) - 34 else 2
                    for _ in range(3):
                        if not pending:
                            break
                        need = (trail_eff + GAP if pending[0][0][2] == 0
                                else trail_eff)
                        if len(pending) > need:
                            pop_one()
                        else:
                            break
                    if bg_work and j >= bg_work[0][0]:
                        bg_work.pop(0)[1]()
                    elif outproj_work and j % 3 == 0:
                        # every 3rd slot: out-proj rides the pss rotation
                        emit_outproj_tile(*outproj_work.pop(0))
                while pending:
                    pop_one()
                    if outproj_work:
                        emit_outproj_tile(*outproj_work.pop(0))
                for _, op in bg_work:
                    op()
                while outproj_work:
                    emit_outproj_tile(*outproj_work.pop(0))

    nc.compile()
    return nc


def _get_nc():
    with _lock:
        if "nc" not in _compiled:
            _compiled["nc"] = _build()
        return _compiled["nc"]


def _chan_perm():
    # fp8 DoubleRow layout: channel (p, i) <- head p//32, d = i*32 + p%32
    cols = np.empty((2, 96), np.int64)
    for i in range(2):
        for p in range(96):
            cols[i, p] = (p // 32) * 64 + i * 32 + (p % 32)
    return cols.reshape(-1)  # j = i*96 + p


def _prep_in_maps(query, key, value, prompt, Wq, bq, Wk, bk, Wv, bv, Wo, bo):
    f32 = np.float32
    qT = [np.ascontiguousarray(query[b].T).astype(BF16) for b in range(B)]
    kT = [np.ascontiguousarray(key[b].T).astype(BF16) for b in range(B)]
    vT = [np.ascontiguousarray(value[b].T).astype(BF16) for b in range(B)]
    perm = _chan_perm()
    in_maps = []
    for core in range(NCORES):
        b, g = core // NG, core % NG
        cs = slice(g * CL, (g + 1) * CL)
        Wq_g = np.asarray(Wq)[cs, :]
        Wk_g = np.asarray(Wk)[cs, :]
        bq_g = np.asarray(bq)[cs].astype(f32)
        bk_g = np.asarray(bk)[cs].astype(f32)
        kp = np.zeros((96, 2, PP), FP8)
        for i in range(2):
            for p in range(96):
                gh = g * HL + p // 32
                d = i * 32 + p % 32
                kp[p, i, :] = prompt[b, 0, :, gh, d].astype(FP8)
        vpa = np.zeros((PP, HL, D + 1), BF16)
        vpa[:, :, D] = 1.0
        for h in range(HL):
            gh = g * HL + h
            vpa[:, h, 0:D] = prompt[b, 1, :, gh, :].astype(BF16)
        in_maps.append({
            "xqT": qT[b], "xkT": kT[b], "xvT": vT[b],
            "wqT": np.ascontiguousarray(Wq_g[perm, :].T).astype(BF16),
            "wkT": np.ascontiguousarray(Wk_g[perm, :].T).astype(BF16),
            "wvT": np.ascontiguousarray(np.asarray(Wv)[cs, :].T).astype(BF16),
            "woT": np.ascontiguousarray(np.asarray(Wo)[:, cs].T).astype(BF16),
            "bq": np.ascontiguousarray(
                bq_g[perm].reshape(2, 96).T).astype(f32),
            "bk": np.ascontiguousarray(
                bk_g[perm].reshape(2, 96).T).astype(f32),
            "bv": np.ascontiguousarray(
                np.asarray(bv)[cs]).astype(f32).reshape(1, CL),
            "kp8": kp, "vp": vpa,
        })
    return in_maps


def _combine(results, bo):
    out = np.empty((B, S, E), np.float32)
    for b in range(B):
        acc = results[b * NG]["outT"].astype(np.float32)
        for g in range(1, NG):
            acc = acc + results[b * NG + g]["outT"]
        out[b] = acc.T
    if bo is not None and np.any(bo):
        out += np.asarray(bo, np.float32)
    return out


def run(inputs, trace=False):
    """Returns (output, exec_time_ns or None)."""
    from concourse import bass_utils

    nc = _get_nc()
    in_maps = _prep_in_maps(**{k: np.asarray(v) for k, v in inputs.items()})
    bo = np.asarray(inputs["bo"])
    res = bass_utils.run_bass_kernel_spmd(
        nc, in_maps, core_ids=list(range(NCORES)), trace=trace,
    )
    return _combine(res.results, bo), res.exec_time_ns


def kernel(**inputs):
    out, _ = run(inputs)
    return out
